# revision 35
# baseline (speedup 1.0000x reference)
"""Trainium2 Bass kernel for the additive-attention + GRU decoder.

Math (per reference):
  feats: [C=512, B=128, T=256] f32
  fp = einsum('cbt,hc->bth', feats, Wi2h)            (hoisted, step-independent)
  32 steps of:
    hp = h @ Wh2h.T + bh2h                           [B, H]
    e = tanh(fp + hp[:, None, :]) @ w_score          [B, T]
    alpha = softmax(e, axis=1)
    ctx = einsum('cbt,bt->bc', feats, alpha)         [B, C]
    GRU(ctx, h) -> h                                  (PyTorch gate order r,z,n)
  probs = stack(h per step, per batch) @ Wgen.T + bgen   [B*32, 96]

Distribution: data-parallel over batch, 16 batches per core on 8 cores.

Key structure (v3):
  - The per-step tanh volume [B,T,H] is the ACT-engine bottleneck. Split it:
    slabs ht=0,1 classic (Pool broadcast-add fp+hp, ACT tanh); slabs ht=2,3
    via the tanh addition formula tanh(a+b) = (A+B)/(1+A*B) with A=tanh(fp)
    hoisted to the prologue and B=tanh(hp) tiny per step, evaluated by a
    fused custom DVE op in ONE pass: 1/(1+AB) from the BITWISE_NOT
    exponent-flip seed (v = -D*bitcast(~D) lands in [4,4.5];
    y = nD*(-8.5-D*nD) = v*(8.5-v)/D = 18.03125/D, +-0.17%).  18.03125 is
    folded into the ht=2,3 columns of w_score host-side.  B is pre-scaled
    by (1-2^-11) so D >= 2^-11 strictly (no ~0 NaN).
  - Batch split in two halves (8+8), software-pipelined half a step apart;
    each phase's "head" (hp/gh matmuls + B=tanh(hp)) is emitted one phase
    early so no engine queues it behind the other half's serial tail.
  - All biases enter as K=1 bias-row matmuls (stationary [1,128] bias rows,
    moving an all-ones column), so there are no bias-add evacuations; the
    GRU gate inputs are read straight out of PSUM.
  - h state is f32; Wh2h/Whh/Wgen stationaries f32, so h needs no f16 copy.
  - softmax denominator: ones[128,128] stationary matmul broadcasts the
    partition-sum to all partitions in one matmul; reciprocal lands [128,B].
  - PSUM: one 4-bank pool per half; all accumulation groups of that half
    rotate through the banks in program order (start=True zeroes a bank).
"""

import numpy as np

C = 512
B_FULL = 128
T = 256
H = 512
S = 32
CLS = 96
NCORES = 8
B = B_FULL // NCORES  # 16 batches per core
HT = H // 128  # 4
CT = C // 128  # 4
TT = T // 128  # 2
G3 = 3 * H  # 1536
MT3 = G3 // 128  # 12
NH = 2  # pipelined batch halves
BH = B // NH  # 8
KREC = 18.03125  # recip-seed scale, folded into wsc cols 2,3
ASCALE = 1.0 - 2.0 ** -11  # keeps D = 1 + A*(ASCALE*B) >= 2^-11

_CACHE = {}


def _register_tanh_sum():
    """Register the fused (A+B)*seedrecip(1+A*B) custom DVE op at runtime.

    Exactly 8 ALU ops (the datapath limit):
      m=A*B; D=m+1; nD=~D; t=D*nD; u=C1-t; y=nD*u; s=A+B; out=y*s
    out = 18.03125*tanh(a+b) for A=tanh(a), B=tanh(b) (C1=-8.5).
    """
    import concourse.dve_ops as dve_ops

    for op in dve_ops.OPS:
        if op.name == "TANH_SUM_ANT":
            return op
    from concourse.dve_spec import (
        AluOp,
        Bin,
        C1,
        One,
        Spec,
        Src0,
        Src1,
        _has_src1,
        lower,
    )
    from concourse.dve_uop import DveOpSpec

    m = Src0 * Src1
    D = m + One
    nD = Bin(AluOp.BITWISE_NOT, D, D)
    t = D * nD
    u = C1 - t
    y1 = nD * u
    body = y1 * (Src0 + Src1)

    def _ref(in0, in1, s0, s1, imm2):
        a = np.asarray(in0, np.float32)
        b = np.broadcast_to(np.asarray(in1, np.float32), a.shape).astype(np.float32)
        mm = a * b
        Dd = (mm + np.float32(1.0)).astype(np.float32)
        nDd = (~Dd.view(np.int32)).view(np.float32)
        tt_ = Dd * nDd
        uu = np.float32(s1) - tt_
        yy = nDd * uu
        return yy * (a + b)

    spec = Spec(body=body, reference=_ref)
    row = dve_ops._CUSTOM_DVE_ROW_BASE + len(dve_ops.OPS)
    shas = {}
    for ver in ("v3", "v4"):
        uops = lower(spec, ver=ver)
        shas[ver] = DveOpSpec(
            name="TANH_SUM_ANT", uops=uops, opcode=row, rd1_en=_has_src1(spec)
        ).sha(ver)
    op = dve_ops.DveOp("TANH_SUM_ANT", spec, subdim=False, uops_sha=shas)
    dve_ops.OPS.append(op)
    dve_ops.CUSTOM_DVE_SPECS[op.name] = spec
    dve_ops._SUB_OPCODE_FOR_NAME[op.name] = row
    return op


def build_nc(n_steps=S):
    import concourse.bass as bass
    import concourse.tile as tile
    from concourse import bacc, mybir

    f16 = mybir.dt.float16
    f32 = mybir.dt.float32
    AF = mybir.ActivationFunctionType
    OP = mybir.AluOpType
    ts = bass.ts

    TANH_SUM = _register_tanh_sum()

    nc = bacc.Bacc("TRN2", target_bir_lowering=False, debug=False)

    # ---- DRAM I/O (per-core shard shapes) ----
    feats_d = nc.dram_tensor("feats", [CT, 128, T * B], f16, kind="ExternalInput")
    featsT_d = nc.dram_tensor("featsT", [TT, 128, B * C], f16, kind="ExternalInput")
    wi2hT_d = nc.dram_tensor("wi2hT", [CT, 128, H], f16, kind="ExternalInput")
    wh2hT_d = nc.dram_tensor("wh2hT", [HT, 128, H], f32, kind="ExternalInput")
    whhT_d = nc.dram_tensor("whhT", [HT, 128, G3], f32, kind="ExternalInput")
    wihT_d = nc.dram_tensor("wihT", [CT, 128, G3], f16, kind="ExternalInput")
    wgenT_d = nc.dram_tensor("wgenT", [HT, 128, CLS], f32, kind="ExternalInput")
    wsc_d = nc.dram_tensor("wsc", [128, HT], f16, kind="ExternalInput")
    hrow_d = nc.dram_tensor("hrow", [1, H], f32, kind="ExternalInput")
    grow_d = nc.dram_tensor("grow", [1, G3], f32, kind="ExternalInput")
    nrow_d = nc.dram_tensor("nrow", [1, H], f32, kind="ExternalInput")
    bgen_d = nc.dram_tensor("bgen", [1, CLS], f16, kind="ExternalInput")
    probs_d = nc.dram_tensor("probs", [B * S, CLS], f32, kind="ExternalOutput")

    with tile.TileContext(nc, pool_alloc_mode="queue") as tc:
        with tc.tile_pool(name="const", bufs=1) as const:
            sb_featsT = const.tile([128, TT, B * C], f16)
            sb_wh2hT = const.tile([128, HT, H], f32)
            for kt in range(HT):
                nc.sync.dma_start(sb_wh2hT[:, kt, :], wh2hT_d.ap()[kt])
            sb_whhT = const.tile([128, HT, G3], f32)
            for kt in range(HT):
                nc.sync.dma_start(sb_whhT[:, kt, :], whhT_d.ap()[kt])
            sb_wihT = const.tile([128, CT, G3], f16)
            for kt in range(CT):
                nc.sync.dma_start(sb_wihT[:, kt, :], wihT_d.ap()[kt])
            sb_wgenT = const.tile([128, HT, CLS], f32)
            for kt in range(HT):
                nc.sync.dma_start(sb_wgenT[:, kt, :], wgenT_d.ap()[kt])
            sb_wsc = const.tile([128, HT], f16)
            nc.sync.dma_start(sb_wsc, wsc_d.ap())
            sb_hrow = const.tile([1, H], f32)
            nc.sync.dma_start(sb_hrow, hrow_d.ap())
            sb_grow = const.tile([1, G3], f32)
            nc.sync.dma_start(sb_grow, grow_d.ap())
            sb_nrow = const.tile([1, H], f32)
            nc.sync.dma_start(sb_nrow, nrow_d.ap())
            sb_bgen = const.tile([1, CLS], f16)
            nc.sync.dma_start(sb_bgen, bgen_d.ap())

            # featsT is DMA'd last: per HW-DGE queue FIFO order, waiting on it
            # covers every earlier constant DMA.
            for tt in range(TT):
                nc.sync.dma_start(sb_featsT[:, tt, :], featsT_d.ap()[tt])

            sb_onesq = const.tile([128, 128], f16)
            nc.vector.memset(sb_onesq, 1.0)
            sb_ones128 = const.tile([1, 128], f16)
            nc.vector.memset(sb_ones128, 1.0)
            sb_ones32 = const.tile([1, B], f32)
            nc.vector.memset(sb_ones32, 1.0)

            # One "prime" instruction per engine reading featsT so the DMA
            # queue waits land on these tiny instructions alone (ISA caps
            # sync-waits per instruction).
            prime_dve = const.tile([1, 8], f16)
            nc.vector.tensor_copy(prime_dve, sb_featsT[0:1, 0, 0:8])
            prime_act = const.tile([1, 8], f16)
            nc.scalar.copy(prime_act, sb_featsT[0:1, 0, 0:8])
            prime_pool = const.tile([1, 8], f16)
            nc.gpsimd.tensor_copy(prime_pool, sb_featsT[0:1, 0, 0:8])

            sb_fpT = const.tile([128, 2, T * B], f16)  # slabs ht=0,1: fp
            sb_AT = const.tile([128, 2, T * B], f16)  # slabs ht=2,3: tanh(fp)
            sb_hidT = const.tile([128, HT, B * S], f32)  # h history, col b*S+s
            hT0 = const.tile([128, HT, B], f32)
            nc.vector.memset(hT0, 0.0)

            # ---- Prologue: fp = Wi2h @ feats (contract C); A = tanh(fp) ----
            with (
                tc.tile_pool(name="prol", bufs=1) as prol,
                tc.tile_pool(name="prol_ps", bufs=4, space="PSUM") as prol_ps,
            ):
                sb_wi2hT = prol.tile([128, CT, H], f16)
                for kt in range(CT):
                    nc.sync.dma_start(sb_wi2hT[:, kt, :], wi2hT_d.ap()[kt])
                nch = (T * B) // 512  # 8
                for n in range(nch):
                    fch = prol.tile(
                        [128, CT, 512], f16, tag="fch", bufs=2, name=f"fch{n}"
                    )
                    for ct in range(CT):
                        nc.sync.dma_start(
                            fch[:, ct, :], feats_d.ap()[ct][:, ts(n, 512)]
                        )
                    for mt in range(HT):
                        ps = prol_ps.tile([128, 512], f32, tag="pro")
                        for ct in range(CT):
                            nc.tensor.matmul(
                                ps,
                                sb_wi2hT[:, ct, ts(mt, 128)],
                                fch[:, ct, :],
                                start=(ct == 0),
                                stop=(ct == CT - 1),
                            )
                        if mt < 2:
                            nc.vector.tensor_copy(sb_fpT[:, mt, ts(n, 512)], ps)
                        else:
                            nc.scalar.activation(
                                sb_AT[:, mt - 2, ts(n, 512)], ps, AF.Tanh
                            )

            # ---- Steps (two software-pipelined batch halves) ----
            with (
                tc.tile_pool(name="step", bufs=2) as sp,
                tc.tile_pool(name="gbuf", bufs=1) as gp,
                tc.tile_pool(name="ps_h0", bufs=4, space="PSUM") as ps_q0,
                tc.tile_pool(name="ps_h1", bufs=4, space="PSUM") as ps_q1,
            ):
                ps_q = [ps_q0, ps_q1]
                hidT_v = sb_hidT.rearrange("p m (b st) -> p m b st", st=S)
                fpT_v = sb_fpT.rearrange("p m (t b) -> p m t b", b=B)
                AT_v = sb_AT.rearrange("p m (t b) -> p m t b", b=B)

                NPH = n_steps * NH
                head_st = {}  # phase -> (ps_hp01, ps_hp23, ps_gh, Bts, gh_rz, gh_n05)

                def emit_head(p):
                    s, hb = divmod(p, NH)
                    bsl = slice(hb * BH, (hb + 1) * BH)
                    hT = hT0[:, :, bsl] if s == 0 else hidT_v[:, :, bsl, s - 1]
                    psq = ps_q[hb]
                    qt = f"q{hb}"
                    ones_m = sb_ones32[0:1, 0:BH]

                    # hp = Wh2h @ h + bh2h (bias as K=1 row matmul), two psum
                    # tiles of two mt-chunks each; slabs read hp from PSUM.
                    hps = []
                    for half in range(2):
                        ps_hp = psq.tile(
                            [128, 2, BH], f32, tag=qt, name=f"hp{s}_{hb}_{half}"
                        )
                        for k in range(2):
                            mt = 2 * half + k
                            nc.tensor.matmul(
                                ps_hp[:, k, :],
                                sb_hrow[0:1, ts(mt, 128)],
                                ones_m,
                                start=(k == 0),
                                stop=False,
                            )
                            for kt in range(HT):
                                nc.tensor.matmul(
                                    ps_hp[:, k, :],
                                    sb_wh2hT[:, kt, ts(mt, 128)],
                                    hT[:, kt, :],
                                    start=False,
                                    stop=(k == 1 and kt == HT - 1),
                                )
                        hps.append(ps_hp)

                    # B = tanh(hp_{ht2,3}) straight from PSUM, then *(1-2^-11)
                    Bt = sp.tile([128, 2, BH], f16, tag=f"Bt{hb}")
                    nc.scalar.activation(Bt, hps[1], AF.Tanh)
                    Bts = sp.tile([128, 2, BH], f16, tag=f"Bts{hb}")
                    nc.vector.tensor_scalar_mul(Bts, Bt, ASCALE)

                    # gh rz-part accumulates straight into the gi psum tile
                    # (one group spanning head+body; trz later reads PSUM).
                    ps_gi = psq.tile(
                        [128, 2 * HT * BH], f32, tag=qt, name=f"gi{s}_{hb}"
                    )
                    for mt in range(2 * HT):
                        nc.tensor.matmul(
                            ps_gi[:, ts(mt, BH)],
                            sb_grow[0:1, ts(mt, 128)],
                            ones_m,
                            start=(mt == 0),
                            stop=False,
                        )
                    for mt in range(2 * HT):
                        for kt in range(HT):
                            nc.tensor.matmul(
                                ps_gi[:, ts(mt, BH)],
                                sb_whhT[:, kt, ts(mt, 128)],
                                hT[:, kt, :],
                                start=False,
                                stop=False,
                            )
                    # gh n-part (weights and bias pre-scaled by 0.5 host-side)
                    ps_ghn = psq.tile(
                        [128, HT * BH], f32, tag=qt, name=f"ghn{s}_{hb}"
                    )
                    for mt in range(HT):
                        nc.tensor.matmul(
                            ps_ghn[:, ts(mt, BH)],
                            sb_grow[0:1, ts(2 * HT + mt, 128)],
                            ones_m,
                            start=(mt == 0),
                            stop=False,
                        )
                    for mt in range(HT):
                        for kt in range(HT):
                            nc.tensor.matmul(
                                ps_ghn[:, ts(mt, BH)],
                                sb_whhT[:, kt, ts(2 * HT + mt, 128)],
                                hT[:, kt, :],
                                start=False,
                                stop=(mt == HT - 1 and kt == HT - 1),
                            )
                    gh_n05 = sp.tile([128, HT, BH], f32, tag=f"ghn{hb}")
                    nc.scalar.copy(
                        gh_n05,
                        ps_ghn.rearrange("p (m b) -> p m b", b=BH),
                    )

                    # Slabs ht=2,3: fused custom op on DVE (depends only on
                    # Bts -> runs during the previous phase's tail)
                    gc = []
                    for k in range(2):
                        g = gp.tile([128, T * BH], f16, tag=f"gc{hb}", bufs=2)
                        nc.vector._custom_dve(
                            TANH_SUM,
                            out=g.rearrange("p (t b) -> p t b", b=BH),
                            in0=AT_v[:, k, :, bsl],
                            in1=Bts[:, k, :].unsqueeze(1).broadcast_to(
                                [128, T, BH]
                            ),
                            s1=-8.5,
                        )
                        gc.append(g)

                    # Slabs ht=0,1 broadcast-adds (Pool; hp evacuated to
                    # SBUF first - GPSIMD cannot read PSUM)
                    hpT01 = sp.tile([128, 2, BH], f16, tag=f"hpT{hb}")
                    nc.vector.tensor_copy(hpT01, hps[0])
                    gt = []
                    for k in range(2):
                        g = gp.tile([128, T * BH], f16, tag=f"ga{hb}_{k}", bufs=2)
                        nc.gpsimd.tensor_tensor(
                            out=g.rearrange("p (t b) -> p t b", b=BH),
                            in0=fpT_v[:, k, :, bsl],
                            in1=hpT01[:, k, :].unsqueeze(1).broadcast_to(
                                [128, T, BH]
                            ),
                            op=OP.add,
                        )
                        gt.append(g)

                    head_st[p] = (gc, gt, ps_gi, gh_n05)

                def emit_body(p):
                    s, hb = divmod(p, NH)
                    bsl = slice(hb * BH, (hb + 1) * BH)
                    hT = hT0[:, :, bsl] if s == 0 else hidT_v[:, :, bsl, s - 1]
                    psq = ps_q[hb]
                    qt = f"q{hb}"
                    gc, gt, ps_gi, gh_n05 = head_st.pop(p)

                    # ACT tanh in place over the add outputs
                    for g in gt:
                        nc.scalar.activation(g, g, AF.Tanh)

                    # e^T psum [128p(t), tt, b] accumulated over all 4 slabs
                    eT = psq.tile([128, TT, BH], f32, tag=qt, name=f"eT{s}_{hb}")
                    slabs = [(gc[0], 2), (gc[1], 3), (gt[0], 0), (gt[1], 1)]
                    for idx, (slab, ht) in enumerate(slabs):
                        slab_v = slab.rearrange("p (t b) -> p t b", b=BH)
                        for b in range(BH):
                            for tt in range(TT):
                                nc.tensor.matmul(
                                    eT[:, tt, b : b + 1],
                                    slab_v[:, tt * 128 : (tt + 1) * 128, b],
                                    sb_wsc[:, ht : ht + 1],
                                    start=(idx == 0 and b == 0 and tt == 0),
                                    stop=(
                                        idx == 3 and b == BH - 1 and tt == TT - 1
                                    ),
                                )

                    expT = sp.tile([128, TT, BH], f16, tag=f"expT{hb}")
                    nc.scalar.activation(expT, eT, AF.Exp)

                    # softmax denominator broadcast to all partitions in one
                    # matmul (ones stationary), then reciprocal
                    ps_sum = psq.tile([128, BH], f32, tag=qt, name=f"sum{s}_{hb}")
                    for tt in range(TT):
                        nc.tensor.matmul(
                            ps_sum,
                            sb_onesq,
                            expT[:, tt, :],
                            start=(tt == 0),
                            stop=(tt == TT - 1),
                        )
                    recip = sp.tile([128, BH], f32, tag=f"rc{hb}")
                    nc.vector.reciprocal(recip, ps_sum)

                    # ctx (one psum tile, normalized in one evac)
                    ctxT = sp.tile([128, CT, BH], f16, tag=f"ctxT{hb}")
                    ps_ctx = psq.tile(
                        [128, CT, BH], f32, tag=qt, name=f"cx{s}_{hb}"
                    )
                    for cc in range(CT):
                        for b in range(BH):
                            bg = hb * BH + b
                            for tt in range(TT):
                                nc.tensor.matmul(
                                    ps_ctx[:, cc, b : b + 1],
                                    sb_featsT[
                                        :,
                                        tt,
                                        bg * C + cc * 128 : bg * C + (cc + 1) * 128,
                                    ],
                                    expT[:, tt, b : b + 1],
                                    start=(cc == 0 and b == 0 and tt == 0),
                                    stop=(
                                        cc == CT - 1
                                        and b == BH - 1
                                        and tt == TT - 1
                                    ),
                                )
                    nc.vector.tensor_tensor(
                        out=ctxT,
                        in0=ps_ctx,
                        in1=recip.unsqueeze(1).broadcast_to([128, CT, BH]),
                        op=OP.mult,
                    )

                    # gi rz-part continues the ps_gi group; n-part separate
                    ps_gin = psq.tile(
                        [128, HT * BH], f32, tag=qt, name=f"gin{s}_{hb}"
                    )
                    ones_m = sb_ones32[0:1, 0:BH]
                    for mt in range(2 * HT):
                        for kt in range(CT):
                            nc.tensor.matmul(
                                ps_gi[:, ts(mt, BH)],
                                sb_wihT[:, kt, ts(mt, 128)],
                                ctxT[:, kt, :],
                                start=False,
                                stop=(mt == 2 * HT - 1 and kt == CT - 1),
                            )
                    for mt in range(HT):
                        nc.tensor.matmul(
                            ps_gin[:, ts(mt, BH)],
                            sb_nrow[0:1, ts(mt, 128)],
                            ones_m,
                            start=(mt == 0),
                            stop=False,
                        )
                    for mt in range(HT):
                        for kt in range(CT):
                            nc.tensor.matmul(
                                ps_gin[:, ts(mt, BH)],
                                sb_wihT[:, kt, ts(2 * HT + mt, 128)],
                                ctxT[:, kt, :],
                                start=False,
                                stop=(mt == HT - 1 and kt == CT - 1),
                            )

                    # Gates. sigmoid(x) = .5 + .5*tanh(x/2); rz from PSUM
                    trz = sp.tile([128, 2 * HT, BH], f32, tag=f"trz{hb}")
                    nc.scalar.activation(
                        trz,
                        ps_gi.rearrange("p (m b) -> p m b", b=BH),
                        AF.Tanh,
                        scale=0.5,
                    )
                    # t2 = (tr+1) * (0.5*(gh_n+bhh_n)) == r * hn
                    t2 = sp.tile([128, HT, BH], f32, tag=f"t2{hb}")
                    nc.vector.scalar_tensor_tensor(
                        out=t2,
                        in0=trz[:, 0:HT, :],
                        scalar=1.0,
                        in1=gh_n05,
                        op0=OP.add,
                        op1=OP.mult,
                    )
                    n_in = sp.tile([128, HT, BH], f32, tag=f"ni{hb}")
                    nc.vector.tensor_tensor(
                        out=n_in,
                        in0=ps_gin.rearrange("p (m b) -> p m b", b=BH),
                        in1=t2,
                        op=OP.add,
                    )
                    n_g = sp.tile([128, HT, BH], f32, tag=f"ng{hb}")
                    nc.scalar.activation(n_g, n_in, AF.Tanh)
                    # h' = n + 0.5*(tz+1)*(h-n)
                    d = sp.tile([128, HT, BH], f32, tag=f"d{hb}")
                    nc.gpsimd.tensor_sub(d, hT, n_g)
                    m6 = sp.tile([128, HT, BH], f32, tag=f"m6{hb}")
                    nc.vector.scalar_tensor_tensor(
                        out=m6,
                        in0=trz[:, HT : 2 * HT, :],
                        scalar=1.0,
                        in1=d,
                        op0=OP.add,
                        op1=OP.mult,
                    )
                    nc.vector.scalar_tensor_tensor(
                        out=hidT_v[:, :, bsl, s],
                        in0=m6,
                        scalar=0.5,
                        in1=n_g,
                        op0=OP.mult,
                        op1=OP.add,
                    )

                # head(1) is emitted after body(0): the queue position delays
                # half 1 by ~half a step, and the balanced per-phase engine
                # load keeps the two halves anti-phased from then on (the
                # tail of one half hides under the slab block of the other).
                emit_head(0)
                for p in range(NPH):
                    emit_body(p)
                    if p == 0:
                        emit_head(1)
                    if p + 2 < NPH:
                        emit_head(p + 2)

                # ---- Epilogue: probs = hiddens @ Wgen.T + bgen ----
                for rt in range(CT):
                    ps_pr = ps_q0.tile([128, CLS], f32, tag="q0", name=f"pr{rt}")
                    for kt in range(HT):
                        nc.tensor.matmul(
                            ps_pr,
                            sb_hidT[:, kt, ts(rt, 128)],
                            sb_wgenT[:, kt, :],
                            start=(kt == 0),
                            stop=False,
                        )
                    nc.tensor.matmul(
                        ps_pr, sb_ones128, sb_bgen, start=False, stop=True
                    )
                    pr = sp.tile([128, CLS], f32, tag="pr_out")
                    nc.vector.tensor_copy(pr, ps_pr)
                    nc.gpsimd.dma_start(probs_d.ap()[ts(rt, 128)], pr)

    nc.compile()
    return nc


def make_in_maps(feats, Wi2h, Wh2h, bh2h, Wscore, Wih, Whh, bih, bhh, Wgen, bgen):
    """Host-side prep: cast, transpose weights, shard feats over batch."""
    f16 = np.float16
    f32 = np.float32
    feats = np.asarray(feats, f32)
    wsc = np.ascontiguousarray(
        np.asarray(Wscore, np.float64)[0].reshape(HT, 128).T
    )
    wsc[:, 2:4] /= KREC
    bih = np.asarray(bih, f32)
    bhh = np.asarray(bhh, f32)
    grow = np.concatenate([(bih + bhh)[: 2 * H], 0.5 * bhh[2 * H :]]).astype(f32)
    common = {
        "wi2hT": np.ascontiguousarray(np.asarray(Wi2h).T).astype(f16).reshape(CT, 128, H),
        "wh2hT": np.ascontiguousarray(np.asarray(Wh2h).T).astype(f32).reshape(HT, 128, H),
        "whhT": np.ascontiguousarray(
            np.asarray(Whh).T * np.concatenate([np.ones(2 * H), np.full(H, 0.5)])
        ).astype(f32).reshape(HT, 128, G3),
        "wihT": np.ascontiguousarray(np.asarray(Wih).T).astype(f16).reshape(CT, 128, G3),
        "wgenT": np.ascontiguousarray(np.asarray(Wgen).T).astype(f32).reshape(HT, 128, CLS),
        "wsc": wsc.astype(f16),
        "hrow": np.asarray(bh2h, f32).reshape(1, H),
        "grow": grow.reshape(1, G3),
        "nrow": bih[2 * H :].reshape(1, H).astype(f32),
        "bgen": np.asarray(bgen, f32).astype(f16).reshape(1, CLS),
    }
    in_maps = []
    for i in range(NCORES):
        sl = slice(i * B, (i + 1) * B)
        fsh = feats[:, sl, :]  # [512, 16, 256]
        m = dict(common)
        # t-major free layout (col = t*16 + b) for the broadcast-adds
        m["feats"] = (
            np.ascontiguousarray(fsh.transpose(0, 2, 1)).astype(f16).reshape(CT, 128, T * B)
        )
        m["featsT"] = (
            np.ascontiguousarray(fsh.transpose(2, 1, 0)).astype(f16).reshape(TT, 128, B * C)
        )
        in_maps.append(m)
    return in_maps


def _get_nc(n_steps=S):
    k = f"nc{n_steps}"
    if k not in _CACHE:
        _CACHE[k] = build_nc(n_steps)
    return _CACHE[k]


def kernel(
    feats,
    text_length,
    Wi2h,
    Wh2h,
    bh2h,
    Wscore,
    Wih,
    Whh,
    bih,
    bhh,
    Wgen,
    bgen,
    **_ignored,
):
    from concourse import bass_utils

    nc = _get_nc()
    in_maps = make_in_maps(
        feats, Wi2h, Wh2h, bh2h, Wscore, Wih, Whh, bih, bhh, Wgen, bgen
    )
    res = bass_utils.run_bass_kernel_spmd(nc, in_maps, core_ids=list(range(NCORES)))
    out = np.concatenate([r["probs"] for r in res.results], axis=0)
    return out.astype(np.float32)


# revision 39
# speedup vs baseline: 1.0130x; 1.0130x over previous
"""Trainium2 Bass kernel for the additive-attention + GRU decoder.

Math (per reference):
  feats: [C=512, B=128, T=256] f32
  fp = einsum('cbt,hc->bth', feats, Wi2h)            (hoisted, step-independent)
  32 steps of:
    hp = h @ Wh2h.T + bh2h                           [B, H]
    e = tanh(fp + hp[:, None, :]) @ w_score          [B, T]
    alpha = softmax(e, axis=1)
    ctx = einsum('cbt,bt->bc', feats, alpha)         [B, C]
    GRU(ctx, h) -> h                                  (PyTorch gate order r,z,n)
  probs = stack(h per step, per batch) @ Wgen.T + bgen   [B*32, 96]

Distribution: data-parallel over batch, 16 batches per core on 8 cores.

Key structure (v3):
  - The per-step tanh volume [B,T,H] is the ACT-engine bottleneck. Split it:
    slabs ht=0,1 classic (Pool broadcast-add fp+hp, ACT tanh); slabs ht=2,3
    via the tanh addition formula tanh(a+b) = (A+B)/(1+A*B) with A=tanh(fp)
    hoisted to the prologue and B=tanh(hp) tiny per step, evaluated by a
    fused custom DVE op in ONE pass: 1/(1+AB) from the BITWISE_NOT
    exponent-flip seed (v = -D*bitcast(~D) lands in [4,4.5];
    y = nD*(-8.5-D*nD) = v*(8.5-v)/D = 18.03125/D, +-0.17%).  18.03125 is
    folded into the ht=2,3 columns of w_score host-side.  B is pre-scaled
    by (1-2^-11) so D >= 2^-11 strictly (no ~0 NaN).
  - Batch split in two halves (8+8), software-pipelined half a step apart;
    each phase's "head" (hp/gh matmuls + B=tanh(hp)) is emitted one phase
    early so no engine queues it behind the other half's serial tail.
  - All biases enter as K=1 bias-row matmuls (stationary [1,128] bias rows,
    moving an all-ones column), so there are no bias-add evacuations; the
    GRU gate inputs are read straight out of PSUM.
  - h state is f32; Wh2h/Whh/Wgen stationaries f32, so h needs no f16 copy.
  - softmax denominator: ones[128,128] stationary matmul broadcasts the
    partition-sum to all partitions in one matmul; reciprocal lands [128,B].
  - PSUM: one 4-bank pool per half; all accumulation groups of that half
    rotate through the banks in program order (start=True zeroes a bank).
"""

import numpy as np

C = 512
B_FULL = 128
T = 256
H = 512
S = 32
CLS = 96
NCORES = 8
B = B_FULL // NCORES  # 16 batches per core
HT = H // 128  # 4
CT = C // 128  # 4
TT = T // 128  # 2
G3 = 3 * H  # 1536
MT3 = G3 // 128  # 12
NH = 2  # pipelined batch halves
BH = B // NH  # 8
KREC = 18.03125  # recip-seed scale, folded into wsc cols 2,3
ASCALE = 1.0 - 2.0 ** -11  # keeps D = 1 + A*(ASCALE*B) >= 2^-11

_CACHE = {}


def _register_tanh_sum():
    """Register the fused (A+B)*seedrecip(1+A*B) custom DVE op at runtime.

    Exactly 8 ALU ops (the datapath limit):
      m=A*B; D=m+1; nD=~D; t=D*nD; u=C1-t; y=nD*u; s=A+B; out=y*s
    out = 18.03125*tanh(a+b) for A=tanh(a), B=tanh(b) (C1=-8.5).
    """
    import concourse.dve_ops as dve_ops

    for op in dve_ops.OPS:
        if op.name == "TANH_SUM_ANT":
            return op
    from concourse.dve_spec import (
        AluOp,
        Bin,
        C1,
        One,
        Spec,
        Src0,
        Src1,
        _has_src1,
        lower,
    )
    from concourse.dve_uop import DveOpSpec

    m = Src0 * Src1
    D = m + One
    nD = Bin(AluOp.BITWISE_NOT, D, D)
    t = D * nD
    u = C1 - t
    y1 = nD * u
    body = y1 * (Src0 + Src1)

    def _ref(in0, in1, s0, s1, imm2):
        a = np.asarray(in0, np.float32)
        b = np.broadcast_to(np.asarray(in1, np.float32), a.shape).astype(np.float32)
        mm = a * b
        Dd = (mm + np.float32(1.0)).astype(np.float32)
        nDd = (~Dd.view(np.int32)).view(np.float32)
        tt_ = Dd * nDd
        uu = np.float32(s1) - tt_
        yy = nDd * uu
        return yy * (a + b)

    spec = Spec(body=body, reference=_ref)
    row = dve_ops._CUSTOM_DVE_ROW_BASE + len(dve_ops.OPS)
    shas = {}
    for ver in ("v3", "v4"):
        uops = lower(spec, ver=ver)
        shas[ver] = DveOpSpec(
            name="TANH_SUM_ANT", uops=uops, opcode=row, rd1_en=_has_src1(spec)
        ).sha(ver)
    op = dve_ops.DveOp("TANH_SUM_ANT", spec, subdim=False, uops_sha=shas)
    dve_ops.OPS.append(op)
    dve_ops.CUSTOM_DVE_SPECS[op.name] = spec
    dve_ops._SUB_OPCODE_FOR_NAME[op.name] = row
    return op


def build_nc(n_steps=S):
    import concourse.bass as bass
    import concourse.tile as tile
    from concourse import bacc, mybir

    f16 = mybir.dt.float16
    f32 = mybir.dt.float32
    AF = mybir.ActivationFunctionType
    OP = mybir.AluOpType
    ts = bass.ts

    TANH_SUM = _register_tanh_sum()

    nc = bacc.Bacc("TRN2", target_bir_lowering=False, debug=False)

    # ---- DRAM I/O (per-core shard shapes) ----
    feats_d = nc.dram_tensor("feats", [CT, 128, T * B], f16, kind="ExternalInput")
    featsT_d = nc.dram_tensor("featsT", [TT, 128, B * C], f16, kind="ExternalInput")
    wi2hT_d = nc.dram_tensor("wi2hT", [CT, 128, H], f16, kind="ExternalInput")
    wh2hT_d = nc.dram_tensor("wh2hT", [HT, 128, H], f32, kind="ExternalInput")
    whhT_d = nc.dram_tensor("whhT", [HT, 128, G3], f32, kind="ExternalInput")
    wihT_d = nc.dram_tensor("wihT", [CT, 128, G3], f16, kind="ExternalInput")
    wgenT_d = nc.dram_tensor("wgenT", [HT, 128, CLS], f32, kind="ExternalInput")
    wsc_d = nc.dram_tensor("wsc", [128, HT], f16, kind="ExternalInput")
    hrow_d = nc.dram_tensor("hrow", [1, H], f32, kind="ExternalInput")
    grow_d = nc.dram_tensor("grow", [1, G3], f32, kind="ExternalInput")
    nrow_d = nc.dram_tensor("nrow", [1, H], f32, kind="ExternalInput")
    ident_d = nc.dram_tensor("ident", [128, 128], f16, kind="ExternalInput")
    bgen_d = nc.dram_tensor("bgen", [1, CLS], f16, kind="ExternalInput")
    probs_d = nc.dram_tensor("probs", [B * S, CLS], f32, kind="ExternalOutput")

    with tile.TileContext(nc, pool_alloc_mode="queue") as tc:
        with tc.tile_pool(name="const", bufs=1) as const:
            sb_featsT = const.tile([128, TT, B * C], f16)
            sb_wh2hT = const.tile([128, HT, H], f32)
            for kt in range(HT):
                nc.sync.dma_start(sb_wh2hT[:, kt, :], wh2hT_d.ap()[kt])
            sb_whhT = const.tile([128, HT, G3], f32)
            for kt in range(HT):
                nc.sync.dma_start(sb_whhT[:, kt, :], whhT_d.ap()[kt])
            sb_wihT = const.tile([128, CT, G3], f16)
            for kt in range(CT):
                nc.sync.dma_start(sb_wihT[:, kt, :], wihT_d.ap()[kt])
            sb_wgenT = const.tile([128, HT, CLS], f32)
            for kt in range(HT):
                nc.sync.dma_start(sb_wgenT[:, kt, :], wgenT_d.ap()[kt])
            sb_wsc = const.tile([128, HT], f16)
            nc.sync.dma_start(sb_wsc, wsc_d.ap())
            sb_hrow = const.tile([1, H], f32)
            nc.sync.dma_start(sb_hrow, hrow_d.ap())
            sb_grow = const.tile([1, G3], f32)
            nc.sync.dma_start(sb_grow, grow_d.ap())
            sb_nrow = const.tile([1, H], f32)
            nc.sync.dma_start(sb_nrow, nrow_d.ap())
            sb_ident = const.tile([128, 128], f16)
            nc.sync.dma_start(sb_ident, ident_d.ap())
            sb_bgen = const.tile([1, CLS], f16)
            nc.sync.dma_start(sb_bgen, bgen_d.ap())

            # featsT is DMA'd last: per HW-DGE queue FIFO order, waiting on it
            # covers every earlier constant DMA.
            for tt in range(TT):
                nc.sync.dma_start(sb_featsT[:, tt, :], featsT_d.ap()[tt])

            sb_onesq = const.tile([128, 128], f16)
            nc.vector.memset(sb_onesq, 1.0)
            sb_ones128 = const.tile([1, 128], f16)
            nc.vector.memset(sb_ones128, 1.0)
            sb_ones32 = const.tile([1, B], f32)
            nc.vector.memset(sb_ones32, 1.0)
            sb_one = const.tile([128, 1], f32)
            nc.vector.memset(sb_one, 1.0)
            sb_half = const.tile([128, 1], f32)
            nc.vector.memset(sb_half, 0.5)

            # One "prime" instruction per engine reading featsT so the DMA
            # queue waits land on these tiny instructions alone (ISA caps
            # sync-waits per instruction).
            prime_dve = const.tile([1, 8], f16)
            nc.vector.tensor_copy(prime_dve, sb_featsT[0:1, 0, 0:8])
            prime_act = const.tile([1, 8], f16)
            nc.scalar.copy(prime_act, sb_featsT[0:1, 0, 0:8])
            prime_pool = const.tile([1, 8], f16)
            nc.gpsimd.tensor_copy(prime_pool, sb_featsT[0:1, 0, 0:8])

            sb_fpT = const.tile([128, 2, T * B], f16)  # slabs ht=0,1: fp
            sb_AT = const.tile([128, 2, T * B], f16)  # slabs ht=2,3: tanh(fp)
            sb_hidT = const.tile([128, HT, B * S], f32)  # h history, col b*S+s
            hT0 = const.tile([128, HT, B], f32)
            nc.vector.memset(hT0, 0.0)

            # ---- Prologue: fp = Wi2h @ feats (contract C); A = tanh(fp) ----
            with (
                tc.tile_pool(name="prol", bufs=1) as prol,
                tc.tile_pool(name="prol_ps", bufs=4, space="PSUM") as prol_ps,
            ):
                sb_wi2hT = prol.tile([128, CT, H], f16)
                for kt in range(CT):
                    nc.sync.dma_start(sb_wi2hT[:, kt, :], wi2hT_d.ap()[kt])
                nch = (T * B) // 512  # 8
                for n in range(nch):
                    fch = prol.tile(
                        [128, CT, 512], f16, tag="fch", bufs=2, name=f"fch{n}"
                    )
                    for ct in range(CT):
                        nc.sync.dma_start(
                            fch[:, ct, :], feats_d.ap()[ct][:, ts(n, 512)]
                        )
                    for mt in range(HT):
                        ps = prol_ps.tile([128, 512], f32, tag="pro")
                        for ct in range(CT):
                            nc.tensor.matmul(
                                ps,
                                sb_wi2hT[:, ct, ts(mt, 128)],
                                fch[:, ct, :],
                                start=(ct == 0),
                                stop=(ct == CT - 1),
                            )
                        if mt < 2:
                            nc.vector.tensor_copy(sb_fpT[:, mt, ts(n, 512)], ps)
                        else:
                            nc.scalar.activation(
                                sb_AT[:, mt - 2, ts(n, 512)], ps, AF.Tanh
                            )

            # ---- Steps (two software-pipelined batch halves) ----
            with (
                tc.tile_pool(name="step", bufs=2) as sp,
                tc.tile_pool(name="gbuf", bufs=1) as gp,
                tc.tile_pool(name="ps_h0", bufs=4, space="PSUM") as ps_q0,
                tc.tile_pool(name="ps_h1", bufs=4, space="PSUM") as ps_q1,
            ):
                ps_q = [ps_q0, ps_q1]
                hidT_v = sb_hidT.rearrange("p m (b st) -> p m b st", st=S)
                fpT_v = sb_fpT.rearrange("p m (t b) -> p m t b", b=B)
                AT_v = sb_AT.rearrange("p m (t b) -> p m t b", b=B)

                NPH = n_steps * NH
                head_st = {}  # phase -> (ps_hp01, ps_hp23, ps_gh, Bts, gh_rz, gh_n05)

                def emit_head(p):
                    s, hb = divmod(p, NH)
                    bsl = slice(hb * BH, (hb + 1) * BH)
                    hT = hT0[:, :, bsl] if s == 0 else hidT_v[:, :, bsl, s - 1]
                    psq = ps_q[hb]
                    qt = f"q{hb}"
                    ones_m = sb_ones32[0:1, 0:BH]

                    # hp = Wh2h @ h + bh2h (bias as K=1 row matmul), two psum
                    # tiles of two mt-chunks each; slabs read hp from PSUM.
                    hps = []
                    for half in range(2):
                        ps_hp = psq.tile(
                            [128, 2, BH], f32, tag=qt, name=f"hp{s}_{hb}_{half}"
                        )
                        for k in range(2):
                            mt = 2 * half + k
                            nc.tensor.matmul(
                                ps_hp[:, k, :],
                                sb_hrow[0:1, ts(mt, 128)],
                                ones_m,
                                start=(k == 0),
                                stop=False,
                            )
                            for kt in range(HT):
                                nc.tensor.matmul(
                                    ps_hp[:, k, :],
                                    sb_wh2hT[:, kt, ts(mt, 128)],
                                    hT[:, kt, :],
                                    start=False,
                                    stop=(k == 1 and kt == HT - 1),
                                )
                        hps.append(ps_hp)

                    # B = tanh(hp_{ht2,3}) straight from PSUM, then *(1-2^-11)
                    Bt = sp.tile([128, 2, BH], f16, tag=f"Bt{hb}")
                    nc.scalar.activation(Bt, hps[1], AF.Tanh)
                    Bts = sp.tile([128, 2, BH], f16, tag=f"Bts{hb}")
                    nc.vector.tensor_scalar_mul(Bts, Bt, ASCALE)
                    # hp evac for the Pool adds, emitted BEFORE the customs so
                    # the DVE runs this tiny copy first and the classic-slab
                    # chain isn't starved behind 4.4us of custom slabs
                    hpT01 = sp.tile([128, 2, BH], f16, tag=f"hpT{hb}")
                    nc.vector.tensor_copy(hpT01, hps[0])

                    # gh rz-part accumulates straight into the gi psum tile
                    # (one group spanning head+body; trz later reads PSUM).
                    ps_gi = psq.tile(
                        [128, 2 * HT * BH], f32, tag=qt, name=f"gi{s}_{hb}"
                    )
                    for mt in range(2 * HT):
                        nc.tensor.matmul(
                            ps_gi[:, ts(mt, BH)],
                            sb_grow[0:1, ts(mt, 128)],
                            ones_m,
                            start=(mt == 0),
                            stop=False,
                        )
                    for mt in range(2 * HT):
                        for kt in range(HT):
                            nc.tensor.matmul(
                                ps_gi[:, ts(mt, BH)],
                                sb_whhT[:, kt, ts(mt, 128)],
                                hT[:, kt, :],
                                start=False,
                                stop=False,
                            )
                    # gh n-part (weights and bias pre-scaled by 0.5 host-side)
                    ps_ghn = psq.tile(
                        [128, HT * BH], f32, tag=qt, name=f"ghn{s}_{hb}"
                    )
                    for mt in range(HT):
                        nc.tensor.matmul(
                            ps_ghn[:, ts(mt, BH)],
                            sb_grow[0:1, ts(2 * HT + mt, 128)],
                            ones_m,
                            start=(mt == 0),
                            stop=False,
                        )
                    for mt in range(HT):
                        for kt in range(HT):
                            nc.tensor.matmul(
                                ps_ghn[:, ts(mt, BH)],
                                sb_whhT[:, kt, ts(2 * HT + mt, 128)],
                                hT[:, kt, :],
                                start=False,
                                stop=(mt == HT - 1 and kt == HT - 1),
                            )
                    gh_n05 = sp.tile([128, HT, BH], f32, tag=f"ghn{hb}")
                    nc.scalar.copy(
                        gh_n05,
                        ps_ghn.rearrange("p (m b) -> p m b", b=BH),
                    )

                    # Slabs ht=2,3: fused custom op on DVE (depends only on
                    # Bts -> runs during the previous phase's tail)
                    gc = []
                    for k in range(2):
                        g = gp.tile([128, T * BH], f16, tag=f"gc{hb}", bufs=2)
                        nc.vector._custom_dve(
                            TANH_SUM,
                            out=g.rearrange("p (t b) -> p t b", b=BH),
                            in0=AT_v[:, k, :, bsl],
                            in1=Bts[:, k, :].unsqueeze(1).broadcast_to(
                                [128, T, BH]
                            ),
                            s1=-8.5,
                        )
                        gc.append(g)

                    # Slabs ht=0,1 broadcast-adds (Pool)
                    gt = []
                    for k in range(2):
                        g = gp.tile([128, T * BH], f16, tag=f"ga{hb}_{k}", bufs=2)
                        nc.gpsimd.tensor_tensor(
                            out=g.rearrange("p (t b) -> p t b", b=BH),
                            in0=fpT_v[:, k, :, bsl],
                            in1=hpT01[:, k, :].unsqueeze(1).broadcast_to(
                                [128, T, BH]
                            ),
                            op=OP.add,
                        )
                        gt.append(g)

                    head_st[p] = (gc, gt, ps_gi, gh_n05)

                def emit_body(p):
                    s, hb = divmod(p, NH)
                    bsl = slice(hb * BH, (hb + 1) * BH)
                    hT = hT0[:, :, bsl] if s == 0 else hidT_v[:, :, bsl, s - 1]
                    psq = ps_q[hb]
                    qt = f"q{hb}"
                    gc, gt, ps_gi, gh_n05 = head_st.pop(p)

                    # ACT tanh in place over the add outputs
                    for g in gt:
                        nc.scalar.activation(g, g, AF.Tanh)

                    # e^T psum [128p(t), tt, b] accumulated over all 4 slabs
                    eT = psq.tile([128, TT, BH], f32, tag=qt, name=f"eT{s}_{hb}")
                    slabs = [(gc[0], 2), (gc[1], 3), (gt[0], 0), (gt[1], 1)]
                    for idx, (slab, ht) in enumerate(slabs):
                        slab_v = slab.rearrange("p (t b) -> p t b", b=BH)
                        for b in range(BH):
                            for tt in range(TT):
                                nc.tensor.matmul(
                                    eT[:, tt, b : b + 1],
                                    slab_v[:, tt * 128 : (tt + 1) * 128, b],
                                    sb_wsc[:, ht : ht + 1],
                                    start=(idx == 0 and b == 0 and tt == 0),
                                    stop=(
                                        idx == 3 and b == BH - 1 and tt == TT - 1
                                    ),
                                )

                    expT = sp.tile([128, TT, BH], f16, tag=f"expT{hb}")
                    nc.scalar.activation(expT, eT, AF.Exp)

                    # softmax denominator broadcast to all partitions in one
                    # matmul (ones stationary), then reciprocal
                    ps_sum = psq.tile([128, BH], f32, tag=qt, name=f"sum{s}_{hb}")
                    for tt in range(TT):
                        nc.tensor.matmul(
                            ps_sum,
                            sb_onesq,
                            expT[:, tt, :],
                            start=(tt == 0),
                            stop=(tt == TT - 1),
                        )
                    recip = sp.tile([128, BH], f32, tag=f"rc{hb}")
                    nc.vector.reciprocal(recip, ps_sum)

                    # ctx (one psum tile, normalized in one evac)
                    ctxT = sp.tile([128, CT, BH], f16, tag=f"ctxT{hb}")
                    ps_ctx = psq.tile(
                        [128, CT, BH], f32, tag=qt, name=f"cx{s}_{hb}"
                    )
                    for cc in range(CT):
                        for b in range(BH):
                            bg = hb * BH + b
                            for tt in range(TT):
                                nc.tensor.matmul(
                                    ps_ctx[:, cc, b : b + 1],
                                    sb_featsT[
                                        :,
                                        tt,
                                        bg * C + cc * 128 : bg * C + (cc + 1) * 128,
                                    ],
                                    expT[:, tt, b : b + 1],
                                    start=(cc == 0 and b == 0 and tt == 0),
                                    stop=(
                                        cc == CT - 1
                                        and b == BH - 1
                                        and tt == TT - 1
                                    ),
                                )
                    nc.vector.tensor_tensor(
                        out=ctxT,
                        in0=ps_ctx,
                        in1=recip.unsqueeze(1).broadcast_to([128, CT, BH]),
                        op=OP.mult,
                    )

                    # gi rz-part continues the ps_gi group; n-part separate
                    ps_gin = psq.tile(
                        [128, HT * BH], f32, tag=qt, name=f"gin{s}_{hb}"
                    )
                    ones_m = sb_ones32[0:1, 0:BH]
                    for mt in range(2 * HT):
                        for kt in range(CT):
                            nc.tensor.matmul(
                                ps_gi[:, ts(mt, BH)],
                                sb_wihT[:, kt, ts(mt, 128)],
                                ctxT[:, kt, :],
                                start=False,
                                stop=(mt == 2 * HT - 1 and kt == CT - 1),
                            )
                    for mt in range(HT):
                        nc.tensor.matmul(
                            ps_gin[:, ts(mt, BH)],
                            sb_nrow[0:1, ts(mt, 128)],
                            ones_m,
                            start=(mt == 0),
                            stop=False,
                        )
                    for mt in range(HT):
                        for kt in range(CT):
                            nc.tensor.matmul(
                                ps_gin[:, ts(mt, BH)],
                                sb_wihT[:, kt, ts(2 * HT + mt, 128)],
                                ctxT[:, kt, :],
                                start=False,
                                stop=False,
                            )

                    # Gates. sigmoid(x) = .5 + .5*tanh(x/2); rz from PSUM
                    trz = sp.tile([128, 2 * HT, BH], f32, tag=f"trz{hb}")
                    nc.scalar.activation(
                        trz,
                        ps_gi.rearrange("p (m b) -> p m b", b=BH),
                        AF.Tanh,
                        scale=0.5,
                    )
                    # rz1 = trz + 1 (both gates at once, Pool TT)
                    rz1 = sp.tile([128, 2 * HT, BH], f32, tag=f"rz1{hb}")
                    nc.gpsimd.tensor_tensor(
                        out=rz1,
                        in0=trz,
                        in1=sb_one.unsqueeze(1).broadcast_to([128, 2 * HT, BH]),
                        op=OP.add,
                    )
                    # t2 = (tr+1) * (0.5*(gh_n+bhh_n)) == r*hn, in f16; summed
                    # into ps_gin by an identity-stationary matmul so n_g can
                    # read PSUM directly (no DVE adds on the n path).
                    t2f = sp.tile([128, HT, BH], f16, tag=f"t2{hb}")
                    nc.gpsimd.tensor_tensor(
                        out=t2f, in0=rz1[:, 0:HT, :], in1=gh_n05, op=OP.mult
                    )
                    for mt in range(HT):
                        nc.tensor.matmul(
                            ps_gin[:, ts(mt, BH)],
                            sb_ident,
                            t2f[:, mt, :],
                            start=False,
                            stop=(mt == HT - 1),
                        )
                    n_g = sp.tile([128, HT, BH], f32, tag=f"ng{hb}")
                    nc.scalar.activation(
                        n_g, ps_gin.rearrange("p (m b) -> p m b", b=BH), AF.Tanh
                    )
                    # h' = n + 0.5*(tz+1)*(h-n), all Pool TensorTensor
                    zs = sp.tile([128, HT, BH], f32, tag=f"zs{hb}")
                    nc.gpsimd.tensor_tensor(
                        out=zs,
                        in0=rz1[:, HT : 2 * HT, :],
                        in1=sb_half.unsqueeze(1).broadcast_to([128, HT, BH]),
                        op=OP.mult,
                    )
                    d = sp.tile([128, HT, BH], f32, tag=f"d{hb}")
                    nc.gpsimd.tensor_sub(d, hT, n_g)
                    m7 = sp.tile([128, HT, BH], f32, tag=f"m7{hb}")
                    nc.gpsimd.tensor_mul(m7, zs, d)
                    nc.gpsimd.tensor_add(hidT_v[:, :, bsl, s], n_g, m7)

                emit_head(0)
                for p in range(NPH):
                    emit_body(p)
                    if p == 0:
                        emit_head(1)
                    if p + 2 < NPH:
                        emit_head(p + 2)

                # ---- Epilogue: probs = hiddens @ Wgen.T + bgen ----
                for rt in range(CT):
                    ps_pr = ps_q0.tile([128, CLS], f32, tag="q0", name=f"pr{rt}")
                    for kt in range(HT):
                        nc.tensor.matmul(
                            ps_pr,
                            sb_hidT[:, kt, ts(rt, 128)],
                            sb_wgenT[:, kt, :],
                            start=(kt == 0),
                            stop=False,
                        )
                    nc.tensor.matmul(
                        ps_pr, sb_ones128, sb_bgen, start=False, stop=True
                    )
                    pr = sp.tile([128, CLS], f32, tag="pr_out")
                    nc.vector.tensor_copy(pr, ps_pr)
                    nc.gpsimd.dma_start(probs_d.ap()[ts(rt, 128)], pr)

    nc.compile()
    return nc


def make_in_maps(feats, Wi2h, Wh2h, bh2h, Wscore, Wih, Whh, bih, bhh, Wgen, bgen):
    """Host-side prep: cast, transpose weights, shard feats over batch."""
    f16 = np.float16
    f32 = np.float32
    feats = np.asarray(feats, f32)
    wsc = np.ascontiguousarray(
        np.asarray(Wscore, np.float64)[0].reshape(HT, 128).T
    )
    wsc[:, 2:4] /= KREC
    bih = np.asarray(bih, f32)
    bhh = np.asarray(bhh, f32)
    grow = np.concatenate([(bih + bhh)[: 2 * H], 0.5 * bhh[2 * H :]]).astype(f32)
    common = {
        "wi2hT": np.ascontiguousarray(np.asarray(Wi2h).T).astype(f16).reshape(CT, 128, H),
        "wh2hT": np.ascontiguousarray(np.asarray(Wh2h).T).astype(f32).reshape(HT, 128, H),
        "whhT": np.ascontiguousarray(
            np.asarray(Whh).T * np.concatenate([np.ones(2 * H), np.full(H, 0.5)])
        ).astype(f32).reshape(HT, 128, G3),
        "wihT": np.ascontiguousarray(np.asarray(Wih).T).astype(f16).reshape(CT, 128, G3),
        "wgenT": np.ascontiguousarray(np.asarray(Wgen).T).astype(f32).reshape(HT, 128, CLS),
        "wsc": wsc.astype(f16),
        "hrow": np.asarray(bh2h, f32).reshape(1, H),
        "grow": grow.reshape(1, G3),
        "nrow": bih[2 * H :].reshape(1, H).astype(f32),
        "ident": np.eye(128, dtype=f16),
        "bgen": np.asarray(bgen, f32).astype(f16).reshape(1, CLS),
    }
    in_maps = []
    for i in range(NCORES):
        sl = slice(i * B, (i + 1) * B)
        fsh = feats[:, sl, :]  # [512, 16, 256]
        m = dict(common)
        # t-major free layout (col = t*16 + b) for the broadcast-adds
        m["feats"] = (
            np.ascontiguousarray(fsh.transpose(0, 2, 1)).astype(f16).reshape(CT, 128, T * B)
        )
        m["featsT"] = (
            np.ascontiguousarray(fsh.transpose(2, 1, 0)).astype(f16).reshape(TT, 128, B * C)
        )
        in_maps.append(m)
    return in_maps


def _get_nc(n_steps=S):
    k = f"nc{n_steps}"
    if k not in _CACHE:
        _CACHE[k] = build_nc(n_steps)
    return _CACHE[k]


def kernel(
    feats,
    text_length,
    Wi2h,
    Wh2h,
    bh2h,
    Wscore,
    Wih,
    Whh,
    bih,
    bhh,
    Wgen,
    bgen,
    **_ignored,
):
    from concourse import bass_utils

    nc = _get_nc()
    in_maps = make_in_maps(
        feats, Wi2h, Wh2h, bh2h, Wscore, Wih, Whh, bih, bhh, Wgen, bgen
    )
    res = bass_utils.run_bass_kernel_spmd(nc, in_maps, core_ids=list(range(NCORES)))
    out = np.concatenate([r["probs"] for r in res.results], axis=0)
    return out.astype(np.float32)


# revision 42
# speedup vs baseline: 1.0466x; 1.0331x over previous
"""Trainium2 Bass kernel for the additive-attention + GRU decoder.

Math (per reference):
  feats: [C=512, B=128, T=256] f32
  fp = einsum('cbt,hc->bth', feats, Wi2h)            (hoisted, step-independent)
  32 steps of:
    hp = h @ Wh2h.T + bh2h                           [B, H]
    e = tanh(fp + hp[:, None, :]) @ w_score          [B, T]
    alpha = softmax(e, axis=1)
    ctx = einsum('cbt,bt->bc', feats, alpha)         [B, C]
    GRU(ctx, h) -> h                                  (PyTorch gate order r,z,n)
  probs = stack(h per step, per batch) @ Wgen.T + bgen   [B*32, 96]

Distribution: data-parallel over batch, 16 batches per core on 8 cores.

Key structure (v3):
  - The per-step tanh volume [B,T,H] is the ACT-engine bottleneck. Split it:
    slabs ht=0,1 classic (Pool broadcast-add fp+hp, ACT tanh); slabs ht=2,3
    via the tanh addition formula tanh(a+b) = (A+B)/(1+A*B) with A=tanh(fp)
    hoisted to the prologue and B=tanh(hp) tiny per step, evaluated by a
    fused custom DVE op in ONE pass: 1/(1+AB) from the BITWISE_NOT
    exponent-flip seed (v = -D*bitcast(~D) lands in [4,4.5];
    y = nD*(-8.5-D*nD) = v*(8.5-v)/D = 18.03125/D, +-0.17%).  18.03125 is
    folded into the ht=2,3 columns of w_score host-side.  B is pre-scaled
    by (1-2^-11) so D >= 2^-11 strictly (no ~0 NaN).
  - Batch split in two halves (8+8), software-pipelined half a step apart;
    each phase's "head" (hp/gh matmuls + B=tanh(hp)) is emitted one phase
    early so no engine queues it behind the other half's serial tail.
  - All biases enter as K=1 bias-row matmuls (stationary [1,128] bias rows,
    moving an all-ones column), so there are no bias-add evacuations; the
    GRU gate inputs are read straight out of PSUM.
  - h state is f32; Wh2h/Whh/Wgen stationaries f32, so h needs no f16 copy.
  - softmax denominator: ones[128,128] stationary matmul broadcasts the
    partition-sum to all partitions in one matmul; reciprocal lands [128,B].
  - PSUM: one 4-bank pool per half; all accumulation groups of that half
    rotate through the banks in program order (start=True zeroes a bank).
"""

import numpy as np

C = 512
B_FULL = 128
T = 256
H = 512
S = 32
CLS = 96
NCORES = 8
B = B_FULL // NCORES  # 16 batches per core
HT = H // 128  # 4
CT = C // 128  # 4
TT = T // 128  # 2
G3 = 3 * H  # 1536
MT3 = G3 // 128  # 12
NH = 2  # pipelined batch halves
BH = B // NH  # 8
KREC = 18.03125  # recip-seed scale, folded into wsc cols 2,3
ASCALE = 1.0 - 2.0 ** -11  # keeps D = 1 + A*(ASCALE*B) >= 2^-11

_CACHE = {}


def _register_tanh_sum():
    """Register the fused (A+B)*seedrecip(1+A*B) custom DVE op at runtime.

    Exactly 8 ALU ops (the datapath limit):
      m=A*B; D=m+1; nD=~D; t=D*nD; u=C1-t; y=nD*u; s=A+B; out=y*s
    out = 18.03125*tanh(a+b) for A=tanh(a), B=tanh(b) (C1=-8.5).
    """
    import concourse.dve_ops as dve_ops

    for op in dve_ops.OPS:
        if op.name == "TANH_SUM_ANT":
            return op
    from concourse.dve_spec import (
        AluOp,
        Bin,
        C1,
        One,
        Spec,
        Src0,
        Src1,
        _has_src1,
        lower,
    )
    from concourse.dve_uop import DveOpSpec

    m = Src0 * Src1
    D = m + One
    nD = Bin(AluOp.BITWISE_NOT, D, D)
    t = D * nD
    u = C1 - t
    y1 = nD * u
    body = y1 * (Src0 + Src1)

    def _ref(in0, in1, s0, s1, imm2):
        a = np.asarray(in0, np.float32)
        b = np.broadcast_to(np.asarray(in1, np.float32), a.shape).astype(np.float32)
        mm = a * b
        Dd = (mm + np.float32(1.0)).astype(np.float32)
        nDd = (~Dd.view(np.int32)).view(np.float32)
        tt_ = Dd * nDd
        uu = np.float32(s1) - tt_
        yy = nDd * uu
        return yy * (a + b)

    spec = Spec(body=body, reference=_ref)
    row = dve_ops._CUSTOM_DVE_ROW_BASE + len(dve_ops.OPS)
    shas = {}
    for ver in ("v3", "v4"):
        uops = lower(spec, ver=ver)
        shas[ver] = DveOpSpec(
            name="TANH_SUM_ANT", uops=uops, opcode=row, rd1_en=_has_src1(spec)
        ).sha(ver)
    op = dve_ops.DveOp("TANH_SUM_ANT", spec, subdim=False, uops_sha=shas)
    dve_ops.OPS.append(op)
    dve_ops.CUSTOM_DVE_SPECS[op.name] = spec
    dve_ops._SUB_OPCODE_FOR_NAME[op.name] = row
    return op


def build_nc(n_steps=S):
    import concourse.bass as bass
    import concourse.tile as tile
    from concourse import bacc, mybir

    f16 = mybir.dt.float16
    f32 = mybir.dt.float32
    AF = mybir.ActivationFunctionType
    OP = mybir.AluOpType
    ts = bass.ts

    TANH_SUM = _register_tanh_sum()

    nc = bacc.Bacc("TRN2", target_bir_lowering=False, debug=False)

    # ---- DRAM I/O (per-core shard shapes) ----
    feats_d = nc.dram_tensor("feats", [CT, 128, T * B], f16, kind="ExternalInput")
    featsT_d = nc.dram_tensor("featsT", [TT, 128, B * C], f16, kind="ExternalInput")
    wi2hT_d = nc.dram_tensor("wi2hT", [CT, 128, H], f16, kind="ExternalInput")
    wh2hT_d = nc.dram_tensor("wh2hT", [HT, 128, H], f32, kind="ExternalInput")
    whhT_d = nc.dram_tensor("whhT", [HT, 128, G3], f32, kind="ExternalInput")
    wihT_d = nc.dram_tensor("wihT", [CT, 128, G3], f16, kind="ExternalInput")
    wgenT_d = nc.dram_tensor("wgenT", [HT, 128, CLS], f32, kind="ExternalInput")
    wsc_d = nc.dram_tensor("wsc", [128, HT], f16, kind="ExternalInput")
    hrow_d = nc.dram_tensor("hrow", [1, H], f32, kind="ExternalInput")
    grow_d = nc.dram_tensor("grow", [1, G3], f32, kind="ExternalInput")
    nrow_d = nc.dram_tensor("nrow", [1, H], f32, kind="ExternalInput")
    ident_d = nc.dram_tensor("ident", [128, 128], f16, kind="ExternalInput")
    bgen_d = nc.dram_tensor("bgen", [1, CLS], f16, kind="ExternalInput")
    probs_d = nc.dram_tensor("probs", [B * S, CLS], f32, kind="ExternalOutput")

    with tile.TileContext(nc, pool_alloc_mode="queue") as tc:
        with tc.tile_pool(name="const", bufs=1) as const:
            sb_featsT = const.tile([128, TT, B * C], f16)
            sb_wh2hT = const.tile([128, HT, H], f32)
            for kt in range(HT):
                nc.sync.dma_start(sb_wh2hT[:, kt, :], wh2hT_d.ap()[kt])
            sb_whhT = const.tile([128, HT, G3], f32)
            for kt in range(HT):
                nc.sync.dma_start(sb_whhT[:, kt, :], whhT_d.ap()[kt])
            sb_wihT = const.tile([128, CT, G3], f16)
            for kt in range(CT):
                nc.sync.dma_start(sb_wihT[:, kt, :], wihT_d.ap()[kt])
            sb_wgenT = const.tile([128, HT, CLS], f32)
            for kt in range(HT):
                nc.sync.dma_start(sb_wgenT[:, kt, :], wgenT_d.ap()[kt])
            sb_wsc = const.tile([128, HT], f16)
            nc.sync.dma_start(sb_wsc, wsc_d.ap())
            sb_hrow = const.tile([1, H], f32)
            nc.sync.dma_start(sb_hrow, hrow_d.ap())
            sb_grow = const.tile([1, G3], f32)
            nc.sync.dma_start(sb_grow, grow_d.ap())
            sb_nrow = const.tile([1, H], f32)
            nc.sync.dma_start(sb_nrow, nrow_d.ap())
            sb_ident = const.tile([128, 128], f16)
            nc.sync.dma_start(sb_ident, ident_d.ap())
            sb_bgen = const.tile([1, CLS], f16)
            nc.sync.dma_start(sb_bgen, bgen_d.ap())

            # featsT is DMA'd last: per HW-DGE queue FIFO order, waiting on it
            # covers every earlier constant DMA.
            for tt in range(TT):
                nc.sync.dma_start(sb_featsT[:, tt, :], featsT_d.ap()[tt])

            sb_onesq = const.tile([128, 128], f16)
            nc.vector.memset(sb_onesq, 1.0)
            sb_ones128 = const.tile([1, 128], f16)
            nc.vector.memset(sb_ones128, 1.0)
            sb_ones32 = const.tile([1, B], f32)
            nc.vector.memset(sb_ones32, 1.0)
            sb_one = const.tile([128, 1], f32)
            nc.vector.memset(sb_one, 1.0)
            sb_half = const.tile([128, 1], f32)
            nc.vector.memset(sb_half, 0.5)

            # One "prime" instruction per engine reading featsT so the DMA
            # queue waits land on these tiny instructions alone (ISA caps
            # sync-waits per instruction).
            prime_dve = const.tile([1, 8], f16)
            nc.vector.tensor_copy(prime_dve, sb_featsT[0:1, 0, 0:8])
            prime_act = const.tile([1, 8], f16)
            nc.scalar.copy(prime_act, sb_featsT[0:1, 0, 0:8])
            prime_pool = const.tile([1, 8], f16)
            nc.gpsimd.tensor_copy(prime_pool, sb_featsT[0:1, 0, 0:8])

            sb_fpT = const.tile([128, 2, T * B], f16)  # slabs ht=0,1: fp
            sb_AT = const.tile([128, 2, T * B], f16)  # slabs ht=2,3: tanh(fp)
            sb_hidT = const.tile([128, HT, B * S], f32)  # h history, col b*S+s
            hT0 = const.tile([128, HT, B], f32)
            nc.vector.memset(hT0, 0.0)

            # ---- Prologue: fp = Wi2h @ feats (contract C); A = tanh(fp) ----
            with (
                tc.tile_pool(name="prol", bufs=1) as prol,
                tc.tile_pool(name="prol_ps", bufs=4, space="PSUM") as prol_ps,
            ):
                sb_wi2hT = prol.tile([128, CT, H], f16)
                for kt in range(CT):
                    nc.sync.dma_start(sb_wi2hT[:, kt, :], wi2hT_d.ap()[kt])
                nch = (T * B) // 512  # 8
                for n in range(nch):
                    fch = prol.tile(
                        [128, CT, 512], f16, tag="fch", bufs=2, name=f"fch{n}"
                    )
                    for ct in range(CT):
                        nc.sync.dma_start(
                            fch[:, ct, :], feats_d.ap()[ct][:, ts(n, 512)]
                        )
                    for mt in range(HT):
                        ps = prol_ps.tile([128, 512], f32, tag="pro")
                        for ct in range(CT):
                            nc.tensor.matmul(
                                ps,
                                sb_wi2hT[:, ct, ts(mt, 128)],
                                fch[:, ct, :],
                                start=(ct == 0),
                                stop=(ct == CT - 1),
                            )
                        if mt < 2:
                            nc.vector.tensor_copy(sb_fpT[:, mt, ts(n, 512)], ps)
                        else:
                            nc.scalar.activation(
                                sb_AT[:, mt - 2, ts(n, 512)], ps, AF.Tanh
                            )

            # ---- Steps (two software-pipelined batch halves) ----
            with (
                tc.tile_pool(name="step", bufs=2) as sp,
                tc.tile_pool(name="gbuf", bufs=1) as gp,
                tc.tile_pool(name="ps_h0", bufs=4, space="PSUM") as ps_q0,
                tc.tile_pool(name="ps_h1", bufs=4, space="PSUM") as ps_q1,
            ):
                ps_q = [ps_q0, ps_q1]
                hidT_v = sb_hidT.rearrange("p m (b st) -> p m b st", st=S)
                fpT_v = sb_fpT.rearrange("p m (t b) -> p m t b", b=B)
                AT_v = sb_AT.rearrange("p m (t b) -> p m t b", b=B)

                NPH = n_steps * NH
                head_st = {}  # phase -> (ps_hp01, ps_hp23, ps_gh, Bts, gh_rz, gh_n05)

                def emit_head(p):
                    s, hb = divmod(p, NH)
                    bsl = slice(hb * BH, (hb + 1) * BH)
                    hT = hT0[:, :, bsl] if s == 0 else hidT_v[:, :, bsl, s - 1]
                    psq = ps_q[hb]
                    qt = f"q{hb}"
                    ones_m = sb_ones32[0:1, 0:BH]

                    # hp = Wh2h @ h + bh2h (bias as K=1 row matmul), two psum
                    # tiles of two mt-chunks each; slabs read hp from PSUM.
                    hps = []
                    for half in range(2):
                        ps_hp = psq.tile(
                            [128, 2, BH], f32, tag=qt, name=f"hp{s}_{hb}_{half}"
                        )
                        for k in range(2):
                            mt = 2 * half + k
                            nc.tensor.matmul(
                                ps_hp[:, k, :],
                                sb_hrow[0:1, ts(mt, 128)],
                                ones_m,
                                start=(k == 0),
                                stop=False,
                            )
                            for kt in range(HT):
                                nc.tensor.matmul(
                                    ps_hp[:, k, :],
                                    sb_wh2hT[:, kt, ts(mt, 128)],
                                    hT[:, kt, :],
                                    start=False,
                                    stop=(k == 1 and kt == HT - 1),
                                )
                        hps.append(ps_hp)

                    # B = tanh(hp_{ht2,3}) straight from PSUM, then *(1-2^-11)
                    Bt = sp.tile([128, 2, BH], f16, tag=f"Bt{hb}")
                    nc.scalar.activation(Bt, hps[1], AF.Tanh)
                    Bts = sp.tile([128, 2, BH], f16, tag=f"Bts{hb}")
                    nc.vector.tensor_scalar_mul(Bts, Bt, ASCALE)
                    # hp evac for the Pool adds, emitted BEFORE the customs so
                    # the DVE runs this tiny copy first and the classic-slab
                    # chain isn't starved behind 4.4us of custom slabs
                    hpT01 = sp.tile([128, 2, BH], f16, tag=f"hpT{hb}")
                    nc.vector.tensor_copy(hpT01, hps[0])

                    # gh rz-part accumulates straight into the gi psum tile
                    # (one group spanning head+body; trz later reads PSUM).
                    ps_gi = psq.tile(
                        [128, 2 * HT * BH], f32, tag=qt, name=f"gi{s}_{hb}"
                    )
                    for mt in range(2 * HT):
                        nc.tensor.matmul(
                            ps_gi[:, ts(mt, BH)],
                            sb_grow[0:1, ts(mt, 128)],
                            ones_m,
                            start=(mt == 0),
                            stop=False,
                        )
                    for mt in range(2 * HT):
                        for kt in range(HT):
                            nc.tensor.matmul(
                                ps_gi[:, ts(mt, BH)],
                                sb_whhT[:, kt, ts(mt, 128)],
                                hT[:, kt, :],
                                start=False,
                                stop=False,
                            )
                    # gh n-part (weights and bias pre-scaled by 0.5 host-side)
                    ps_ghn = psq.tile(
                        [128, HT * BH], f32, tag=qt, name=f"ghn{s}_{hb}"
                    )
                    for mt in range(HT):
                        nc.tensor.matmul(
                            ps_ghn[:, ts(mt, BH)],
                            sb_grow[0:1, ts(2 * HT + mt, 128)],
                            ones_m,
                            start=(mt == 0),
                            stop=False,
                        )
                    for mt in range(HT):
                        for kt in range(HT):
                            nc.tensor.matmul(
                                ps_ghn[:, ts(mt, BH)],
                                sb_whhT[:, kt, ts(2 * HT + mt, 128)],
                                hT[:, kt, :],
                                start=False,
                                stop=(mt == HT - 1 and kt == HT - 1),
                            )
                    gh_n05 = sp.tile([128, HT, BH], f32, tag=f"ghn{hb}")
                    nc.scalar.copy(
                        gh_n05,
                        ps_ghn.rearrange("p (m b) -> p m b", b=BH),
                    )

                    # Slabs ht=2,3: fused custom op on DVE (depends only on
                    # Bts -> runs during the previous phase's tail)
                    gc = []
                    for k in range(2):
                        g = gp.tile([128, T * BH], f16, tag=f"gc{hb}", bufs=2)
                        nc.vector._custom_dve(
                            TANH_SUM,
                            out=g.rearrange("p (t b) -> p t b", b=BH),
                            in0=AT_v[:, k, :, bsl],
                            in1=Bts[:, k, :].unsqueeze(1).broadcast_to(
                                [128, T, BH]
                            ),
                            s1=-8.5,
                        )
                        gc.append(g)

                    # Slabs ht=0,1 broadcast-adds (Pool)
                    gt = []
                    for k in range(2):
                        g = gp.tile([128, T * BH], f16, tag=f"ga{hb}_{k}", bufs=2)
                        gv = g.rearrange("p (t b) -> p t b", b=BH)
                        for th in range(2):
                            tsl = slice(th * 128, (th + 1) * 128)
                            nc.gpsimd.tensor_tensor(
                                out=gv[:, tsl, :],
                                in0=fpT_v[:, k, tsl, bsl],
                                in1=hpT01[:, k, :].unsqueeze(1).broadcast_to(
                                    [128, 128, BH]
                                ),
                                op=OP.add,
                            )
                        gt.append(g)

                    head_st[p] = (gc, gt, ps_gi, gh_n05)

                def emit_body(p):
                    s, hb = divmod(p, NH)
                    bsl = slice(hb * BH, (hb + 1) * BH)
                    hT = hT0[:, :, bsl] if s == 0 else hidT_v[:, :, bsl, s - 1]
                    psq = ps_q[hb]
                    qt = f"q{hb}"
                    gc, gt, ps_gi, gh_n05 = head_st.pop(p)

                    # ACT tanh in place over the add outputs, in t-halves so
                    # small ACT ops (Bt/exp) aren't stuck behind 1.9us slabs
                    # and the tt=0 eT bursts can start after the first half
                    for g in gt:
                        h2 = T * BH // 2
                        nc.scalar.activation(g[:, 0:h2], g[:, 0:h2], AF.Tanh)
                        nc.scalar.activation(g[:, h2:], g[:, h2:], AF.Tanh)

                    # e^T psum [128p(t), tt, b] accumulated over all 4 slabs
                    eT = psq.tile([128, TT, BH], f32, tag=qt, name=f"eT{s}_{hb}")
                    slabs = [(gc[0], 2), (gc[1], 3), (gt[0], 0), (gt[1], 1)]
                    for idx, (slab, ht) in enumerate(slabs):
                        slab_v = slab.rearrange("p (t b) -> p t b", b=BH)
                        for tt in range(TT):
                            for b in range(BH):
                                nc.tensor.matmul(
                                    eT[:, tt, b : b + 1],
                                    slab_v[:, tt * 128 : (tt + 1) * 128, b],
                                    sb_wsc[:, ht : ht + 1],
                                    start=(idx == 0 and b == 0 and tt == 0),
                                    stop=(
                                        idx == 3 and b == BH - 1 and tt == TT - 1
                                    ),
                                )

                    expT = sp.tile([128, TT, BH], f16, tag=f"expT{hb}")
                    nc.scalar.activation(expT, eT, AF.Exp)

                    # softmax denominator broadcast to all partitions in one
                    # matmul (ones stationary), then reciprocal
                    ps_sum = psq.tile([128, BH], f32, tag=qt, name=f"sum{s}_{hb}")
                    for tt in range(TT):
                        nc.tensor.matmul(
                            ps_sum,
                            sb_onesq,
                            expT[:, tt, :],
                            start=(tt == 0),
                            stop=(tt == TT - 1),
                        )
                    recip = sp.tile([128, BH], f32, tag=f"rc{hb}")
                    nc.vector.reciprocal(recip, ps_sum)

                    # ctx (one psum tile, normalized in one evac)
                    ctxT = sp.tile([128, CT, BH], f16, tag=f"ctxT{hb}")
                    ps_ctx = psq.tile(
                        [128, CT, BH], f32, tag=qt, name=f"cx{s}_{hb}"
                    )
                    for cc in range(CT):
                        for b in range(BH):
                            bg = hb * BH + b
                            for tt in range(TT):
                                nc.tensor.matmul(
                                    ps_ctx[:, cc, b : b + 1],
                                    sb_featsT[
                                        :,
                                        tt,
                                        bg * C + cc * 128 : bg * C + (cc + 1) * 128,
                                    ],
                                    expT[:, tt, b : b + 1],
                                    start=(cc == 0 and b == 0 and tt == 0),
                                    stop=(
                                        cc == CT - 1
                                        and b == BH - 1
                                        and tt == TT - 1
                                    ),
                                )
                    nc.vector.tensor_tensor(
                        out=ctxT,
                        in0=ps_ctx,
                        in1=recip.unsqueeze(1).broadcast_to([128, CT, BH]),
                        op=OP.mult,
                    )

                    # gi rz-part continues the ps_gi group; n-part separate
                    ps_gin = psq.tile(
                        [128, HT * BH], f32, tag=qt, name=f"gin{s}_{hb}"
                    )
                    ones_m = sb_ones32[0:1, 0:BH]
                    for mt in range(2 * HT):
                        for kt in range(CT):
                            nc.tensor.matmul(
                                ps_gi[:, ts(mt, BH)],
                                sb_wihT[:, kt, ts(mt, 128)],
                                ctxT[:, kt, :],
                                start=False,
                                stop=(mt == 2 * HT - 1 and kt == CT - 1),
                            )
                    for mt in range(HT):
                        nc.tensor.matmul(
                            ps_gin[:, ts(mt, BH)],
                            sb_nrow[0:1, ts(mt, 128)],
                            ones_m,
                            start=(mt == 0),
                            stop=False,
                        )
                    for mt in range(HT):
                        for kt in range(CT):
                            nc.tensor.matmul(
                                ps_gin[:, ts(mt, BH)],
                                sb_wihT[:, kt, ts(2 * HT + mt, 128)],
                                ctxT[:, kt, :],
                                start=False,
                                stop=False,
                            )

                    # Gates. sigmoid(x) = .5 + .5*tanh(x/2); rz from PSUM
                    trz = sp.tile([128, 2 * HT, BH], f32, tag=f"trz{hb}")
                    nc.scalar.activation(
                        trz,
                        ps_gi.rearrange("p (m b) -> p m b", b=BH),
                        AF.Tanh,
                        scale=0.5,
                    )
                    # rz1 = trz + 1 (both gates at once, Pool TT)
                    rz1 = sp.tile([128, 2 * HT, BH], f32, tag=f"rz1{hb}")
                    nc.gpsimd.tensor_tensor(
                        out=rz1,
                        in0=trz,
                        in1=sb_one.unsqueeze(1).broadcast_to([128, 2 * HT, BH]),
                        op=OP.add,
                    )
                    # t2 = (tr+1) * (0.5*(gh_n+bhh_n)) == r*hn, in f16; summed
                    # into ps_gin by an identity-stationary matmul so n_g can
                    # read PSUM directly (no DVE adds on the n path).
                    t2f = sp.tile([128, HT, BH], f16, tag=f"t2{hb}")
                    nc.gpsimd.tensor_tensor(
                        out=t2f, in0=rz1[:, 0:HT, :], in1=gh_n05, op=OP.mult
                    )
                    for mt in range(HT):
                        nc.tensor.matmul(
                            ps_gin[:, ts(mt, BH)],
                            sb_ident,
                            t2f[:, mt, :],
                            start=False,
                            stop=(mt == HT - 1),
                        )
                    n_g = sp.tile([128, HT, BH], f32, tag=f"ng{hb}")
                    nc.scalar.activation(
                        n_g, ps_gin.rearrange("p (m b) -> p m b", b=BH), AF.Tanh
                    )
                    # h' = n + 0.5*(tz+1)*(h-n), all Pool TensorTensor
                    zs = sp.tile([128, HT, BH], f32, tag=f"zs{hb}")
                    nc.gpsimd.tensor_tensor(
                        out=zs,
                        in0=rz1[:, HT : 2 * HT, :],
                        in1=sb_half.unsqueeze(1).broadcast_to([128, HT, BH]),
                        op=OP.mult,
                    )
                    d = sp.tile([128, HT, BH], f32, tag=f"d{hb}")
                    nc.gpsimd.tensor_sub(d, hT, n_g)
                    m7 = sp.tile([128, HT, BH], f32, tag=f"m7{hb}")
                    nc.gpsimd.tensor_mul(m7, zs, d)
                    nc.gpsimd.tensor_add(hidT_v[:, :, bsl, s], n_g, m7)

                emit_head(0)
                for p in range(NPH):
                    emit_body(p)
                    if p == 0:
                        emit_head(1)
                    if p + 2 < NPH:
                        emit_head(p + 2)

                # ---- Epilogue: probs = hiddens @ Wgen.T + bgen ----
                for rt in range(CT):
                    ps_pr = ps_q0.tile([128, CLS], f32, tag="q0", name=f"pr{rt}")
                    for kt in range(HT):
                        nc.tensor.matmul(
                            ps_pr,
                            sb_hidT[:, kt, ts(rt, 128)],
                            sb_wgenT[:, kt, :],
                            start=(kt == 0),
                            stop=False,
                        )
                    nc.tensor.matmul(
                        ps_pr, sb_ones128, sb_bgen, start=False, stop=True
                    )
                    pr = sp.tile([128, CLS], f32, tag="pr_out")
                    nc.vector.tensor_copy(pr, ps_pr)
                    nc.gpsimd.dma_start(probs_d.ap()[ts(rt, 128)], pr)

    nc.compile()
    return nc


def make_in_maps(feats, Wi2h, Wh2h, bh2h, Wscore, Wih, Whh, bih, bhh, Wgen, bgen):
    """Host-side prep: cast, transpose weights, shard feats over batch."""
    f16 = np.float16
    f32 = np.float32
    feats = np.asarray(feats, f32)
    wsc = np.ascontiguousarray(
        np.asarray(Wscore, np.float64)[0].reshape(HT, 128).T
    )
    wsc[:, 2:4] /= KREC
    bih = np.asarray(bih, f32)
    bhh = np.asarray(bhh, f32)
    grow = np.concatenate([(bih + bhh)[: 2 * H], 0.5 * bhh[2 * H :]]).astype(f32)
    common = {
        "wi2hT": np.ascontiguousarray(np.asarray(Wi2h).T).astype(f16).reshape(CT, 128, H),
        "wh2hT": np.ascontiguousarray(np.asarray(Wh2h).T).astype(f32).reshape(HT, 128, H),
        "whhT": np.ascontiguousarray(
            np.asarray(Whh).T * np.concatenate([np.ones(2 * H), np.full(H, 0.5)])
        ).astype(f32).reshape(HT, 128, G3),
        "wihT": np.ascontiguousarray(np.asarray(Wih).T).astype(f16).reshape(CT, 128, G3),
        "wgenT": np.ascontiguousarray(np.asarray(Wgen).T).astype(f32).reshape(HT, 128, CLS),
        "wsc": wsc.astype(f16),
        "hrow": np.asarray(bh2h, f32).reshape(1, H),
        "grow": grow.reshape(1, G3),
        "nrow": bih[2 * H :].reshape(1, H).astype(f32),
        "ident": np.eye(128, dtype=f16),
        "bgen": np.asarray(bgen, f32).astype(f16).reshape(1, CLS),
    }
    in_maps = []
    for i in range(NCORES):
        sl = slice(i * B, (i + 1) * B)
        fsh = feats[:, sl, :]  # [512, 16, 256]
        m = dict(common)
        # t-major free layout (col = t*16 + b) for the broadcast-adds
        m["feats"] = (
            np.ascontiguousarray(fsh.transpose(0, 2, 1)).astype(f16).reshape(CT, 128, T * B)
        )
        m["featsT"] = (
            np.ascontiguousarray(fsh.transpose(2, 1, 0)).astype(f16).reshape(TT, 128, B * C)
        )
        in_maps.append(m)
    return in_maps


def _get_nc(n_steps=S):
    k = f"nc{n_steps}"
    if k not in _CACHE:
        _CACHE[k] = build_nc(n_steps)
    return _CACHE[k]


def kernel(
    feats,
    text_length,
    Wi2h,
    Wh2h,
    bh2h,
    Wscore,
    Wih,
    Whh,
    bih,
    bhh,
    Wgen,
    bgen,
    **_ignored,
):
    from concourse import bass_utils

    nc = _get_nc()
    in_maps = make_in_maps(
        feats, Wi2h, Wh2h, bh2h, Wscore, Wih, Whh, bih, bhh, Wgen, bgen
    )
    res = bass_utils.run_bass_kernel_spmd(nc, in_maps, core_ids=list(range(NCORES)))
    out = np.concatenate([r["probs"] for r in res.results], axis=0)
    return out.astype(np.float32)


# revision 43
# speedup vs baseline: 1.0664x; 1.0190x over previous
"""Trainium2 Bass kernel for the additive-attention + GRU decoder.

Math (per reference):
  feats: [C=512, B=128, T=256] f32
  fp = einsum('cbt,hc->bth', feats, Wi2h)            (hoisted, step-independent)
  32 steps of:
    hp = h @ Wh2h.T + bh2h                           [B, H]
    e = tanh(fp + hp[:, None, :]) @ w_score          [B, T]
    alpha = softmax(e, axis=1)
    ctx = einsum('cbt,bt->bc', feats, alpha)         [B, C]
    GRU(ctx, h) -> h                                  (PyTorch gate order r,z,n)
  probs = stack(h per step, per batch) @ Wgen.T + bgen   [B*32, 96]

Distribution: data-parallel over batch, 16 batches per core on 8 cores.

Key structure (v3):
  - The per-step tanh volume [B,T,H] is the ACT-engine bottleneck. Split it:
    slabs ht=0,1 classic (Pool broadcast-add fp+hp, ACT tanh); slabs ht=2,3
    via the tanh addition formula tanh(a+b) = (A+B)/(1+A*B) with A=tanh(fp)
    hoisted to the prologue and B=tanh(hp) tiny per step, evaluated by a
    fused custom DVE op in ONE pass: 1/(1+AB) from the BITWISE_NOT
    exponent-flip seed (v = -D*bitcast(~D) lands in [4,4.5];
    y = nD*(-8.5-D*nD) = v*(8.5-v)/D = 18.03125/D, +-0.17%).  18.03125 is
    folded into the ht=2,3 columns of w_score host-side.  B is pre-scaled
    by (1-2^-11) so D >= 2^-11 strictly (no ~0 NaN).
  - Batch split in two halves (8+8), software-pipelined half a step apart;
    each phase's "head" (hp/gh matmuls + B=tanh(hp)) is emitted one phase
    early so no engine queues it behind the other half's serial tail.
  - All biases enter as K=1 bias-row matmuls (stationary [1,128] bias rows,
    moving an all-ones column), so there are no bias-add evacuations; the
    GRU gate inputs are read straight out of PSUM.
  - h state is f32; Wh2h/Whh/Wgen stationaries f32, so h needs no f16 copy.
  - softmax denominator: ones[128,128] stationary matmul broadcasts the
    partition-sum to all partitions in one matmul; reciprocal lands [128,B].
  - PSUM: one 4-bank pool per half; all accumulation groups of that half
    rotate through the banks in program order (start=True zeroes a bank).
"""

import numpy as np

C = 512
B_FULL = 128
T = 256
H = 512
S = 32
CLS = 96
NCORES = 8
B = B_FULL // NCORES  # 16 batches per core
HT = H // 128  # 4
CT = C // 128  # 4
TT = T // 128  # 2
G3 = 3 * H  # 1536
MT3 = G3 // 128  # 12
NH = 2  # pipelined batch halves
BH = B // NH  # 8
KREC = 18.03125  # recip-seed scale, folded into wsc cols 2,3
ASCALE = 1.0 - 2.0 ** -11  # keeps D = 1 + A*(ASCALE*B) >= 2^-11

_CACHE = {}


def _register_tanh_sum():
    """Register the fused (A+B)*seedrecip(1+A*B) custom DVE op at runtime.

    Exactly 8 ALU ops (the datapath limit):
      m=A*B; D=m+1; nD=~D; t=D*nD; u=C1-t; y=nD*u; s=A+B; out=y*s
    out = 18.03125*tanh(a+b) for A=tanh(a), B=tanh(b) (C1=-8.5).
    """
    import concourse.dve_ops as dve_ops

    for op in dve_ops.OPS:
        if op.name == "TANH_SUM_ANT":
            return op
    from concourse.dve_spec import (
        AluOp,
        Bin,
        C1,
        One,
        Spec,
        Src0,
        Src1,
        _has_src1,
        lower,
    )
    from concourse.dve_uop import DveOpSpec

    m = Src0 * Src1
    D = m + One
    nD = Bin(AluOp.BITWISE_NOT, D, D)
    t = D * nD
    u = C1 - t
    y1 = nD * u
    body = y1 * (Src0 + Src1)

    def _ref(in0, in1, s0, s1, imm2):
        a = np.asarray(in0, np.float32)
        b = np.broadcast_to(np.asarray(in1, np.float32), a.shape).astype(np.float32)
        mm = a * b
        Dd = (mm + np.float32(1.0)).astype(np.float32)
        nDd = (~Dd.view(np.int32)).view(np.float32)
        tt_ = Dd * nDd
        uu = np.float32(s1) - tt_
        yy = nDd * uu
        return yy * (a + b)

    spec = Spec(body=body, reference=_ref)
    row = dve_ops._CUSTOM_DVE_ROW_BASE + len(dve_ops.OPS)
    shas = {}
    for ver in ("v3", "v4"):
        uops = lower(spec, ver=ver)
        shas[ver] = DveOpSpec(
            name="TANH_SUM_ANT", uops=uops, opcode=row, rd1_en=_has_src1(spec)
        ).sha(ver)
    op = dve_ops.DveOp("TANH_SUM_ANT", spec, subdim=False, uops_sha=shas)
    dve_ops.OPS.append(op)
    dve_ops.CUSTOM_DVE_SPECS[op.name] = spec
    dve_ops._SUB_OPCODE_FOR_NAME[op.name] = row
    return op


def build_nc(n_steps=S):
    import concourse.bass as bass
    import concourse.tile as tile
    from concourse import bacc, mybir

    f16 = mybir.dt.float16
    f32 = mybir.dt.float32
    AF = mybir.ActivationFunctionType
    OP = mybir.AluOpType
    ts = bass.ts

    TANH_SUM = _register_tanh_sum()

    nc = bacc.Bacc("TRN2", target_bir_lowering=False, debug=False)

    # ---- DRAM I/O (per-core shard shapes) ----
    feats_d = nc.dram_tensor("feats", [CT, 128, T * B], f16, kind="ExternalInput")
    featsT_d = nc.dram_tensor("featsT", [TT, 128, B * C], f16, kind="ExternalInput")
    wi2hT_d = nc.dram_tensor("wi2hT", [CT, 128, H], f16, kind="ExternalInput")
    wh2hT_d = nc.dram_tensor("wh2hT", [HT, 128, H], f32, kind="ExternalInput")
    whhT_d = nc.dram_tensor("whhT", [HT, 128, G3], f32, kind="ExternalInput")
    wihT_d = nc.dram_tensor("wihT", [CT, 128, G3], f16, kind="ExternalInput")
    wgenT_d = nc.dram_tensor("wgenT", [HT, 128, CLS], f32, kind="ExternalInput")
    wsc_d = nc.dram_tensor("wsc", [128, HT], f16, kind="ExternalInput")
    hrow_d = nc.dram_tensor("hrow", [1, H], f32, kind="ExternalInput")
    grow_d = nc.dram_tensor("grow", [1, G3], f32, kind="ExternalInput")
    nrow_d = nc.dram_tensor("nrow", [1, H], f32, kind="ExternalInput")
    ident_d = nc.dram_tensor("ident", [128, 128], f16, kind="ExternalInput")
    bgen_d = nc.dram_tensor("bgen", [1, CLS], f16, kind="ExternalInput")
    probs_d = nc.dram_tensor("probs", [B * S, CLS], f32, kind="ExternalOutput")

    with tile.TileContext(nc, pool_alloc_mode="queue") as tc:
        with tc.tile_pool(name="const", bufs=1) as const:
            sb_featsT = const.tile([128, TT, B * C], f16)
            sb_wh2hT = const.tile([128, HT, H], f32)
            for kt in range(HT):
                nc.sync.dma_start(sb_wh2hT[:, kt, :], wh2hT_d.ap()[kt])
            sb_whhT = const.tile([128, HT, G3], f32)
            for kt in range(HT):
                nc.sync.dma_start(sb_whhT[:, kt, :], whhT_d.ap()[kt])
            sb_wihT = const.tile([128, CT, G3], f16)
            for kt in range(CT):
                nc.sync.dma_start(sb_wihT[:, kt, :], wihT_d.ap()[kt])
            sb_wgenT = const.tile([128, HT, CLS], f32)
            for kt in range(HT):
                nc.sync.dma_start(sb_wgenT[:, kt, :], wgenT_d.ap()[kt])
            sb_wsc = const.tile([128, HT], f16)
            nc.sync.dma_start(sb_wsc, wsc_d.ap())
            sb_hrow = const.tile([1, H], f32)
            nc.sync.dma_start(sb_hrow, hrow_d.ap())
            sb_grow = const.tile([1, G3], f32)
            nc.sync.dma_start(sb_grow, grow_d.ap())
            sb_nrow = const.tile([1, H], f32)
            nc.sync.dma_start(sb_nrow, nrow_d.ap())
            sb_ident = const.tile([128, 128], f16)
            nc.sync.dma_start(sb_ident, ident_d.ap())
            sb_bgen = const.tile([1, CLS], f16)
            nc.sync.dma_start(sb_bgen, bgen_d.ap())

            # featsT is DMA'd last: per HW-DGE queue FIFO order, waiting on it
            # covers every earlier constant DMA.
            for tt in range(TT):
                nc.sync.dma_start(sb_featsT[:, tt, :], featsT_d.ap()[tt])

            sb_onesq = const.tile([128, 128], f16)
            nc.vector.memset(sb_onesq, 1.0)
            sb_ones128 = const.tile([1, 128], f16)
            nc.vector.memset(sb_ones128, 1.0)
            sb_ones32 = const.tile([1, B], f32)
            nc.vector.memset(sb_ones32, 1.0)
            sb_one = const.tile([128, 1], f32)
            nc.vector.memset(sb_one, 1.0)
            sb_half = const.tile([128, 1], f32)
            nc.vector.memset(sb_half, 0.5)

            # One "prime" instruction per engine reading featsT so the DMA
            # queue waits land on these tiny instructions alone (ISA caps
            # sync-waits per instruction).
            prime_dve = const.tile([1, 8], f16)
            nc.vector.tensor_copy(prime_dve, sb_featsT[0:1, 0, 0:8])
            prime_act = const.tile([1, 8], f16)
            nc.scalar.copy(prime_act, sb_featsT[0:1, 0, 0:8])
            prime_pool = const.tile([1, 8], f16)
            nc.gpsimd.tensor_copy(prime_pool, sb_featsT[0:1, 0, 0:8])

            sb_fpT = const.tile([128, 2, T * B], f16)  # slabs ht=0,1: fp
            sb_AT = const.tile([128, 2, T * B], f16)  # slabs ht=2,3: tanh(fp)
            sb_hidT = const.tile([128, HT, B * S], f32)  # h history, col b*S+s
            hT0 = const.tile([128, HT, B], f32)
            nc.vector.memset(hT0, 0.0)

            # ---- Prologue: fp = Wi2h @ feats (contract C); A = tanh(fp) ----
            with (
                tc.tile_pool(name="prol", bufs=1) as prol,
                tc.tile_pool(name="prol_ps", bufs=4, space="PSUM") as prol_ps,
            ):
                sb_wi2hT = prol.tile([128, CT, H], f16)
                for kt in range(CT):
                    nc.sync.dma_start(sb_wi2hT[:, kt, :], wi2hT_d.ap()[kt])
                nch = (T * B) // 512  # 8
                for n in range(nch):
                    fch = prol.tile(
                        [128, CT, 512], f16, tag="fch", bufs=2, name=f"fch{n}"
                    )
                    for ct in range(CT):
                        nc.sync.dma_start(
                            fch[:, ct, :], feats_d.ap()[ct][:, ts(n, 512)]
                        )
                    for mt in range(HT):
                        ps = prol_ps.tile([128, 512], f32, tag="pro")
                        for ct in range(CT):
                            nc.tensor.matmul(
                                ps,
                                sb_wi2hT[:, ct, ts(mt, 128)],
                                fch[:, ct, :],
                                start=(ct == 0),
                                stop=(ct == CT - 1),
                            )
                        if mt < 2:
                            nc.vector.tensor_copy(sb_fpT[:, mt, ts(n, 512)], ps)
                        else:
                            nc.scalar.activation(
                                sb_AT[:, mt - 2, ts(n, 512)], ps, AF.Tanh
                            )

            # ---- Steps (two software-pipelined batch halves) ----
            with (
                tc.tile_pool(name="step", bufs=2) as sp,
                tc.tile_pool(name="gbuf", bufs=1) as gp,
                tc.tile_pool(name="ps_h0", bufs=4, space="PSUM") as ps_q0,
                tc.tile_pool(name="ps_h1", bufs=4, space="PSUM") as ps_q1,
            ):
                ps_q = [ps_q0, ps_q1]
                hidT_v = sb_hidT.rearrange("p m (b st) -> p m b st", st=S)
                fpT_v = sb_fpT.rearrange("p m (t b) -> p m t b", b=B)
                AT_v = sb_AT.rearrange("p m (t b) -> p m t b", b=B)

                NPH = n_steps * NH
                head_st = {}  # phase -> (ps_hp01, ps_hp23, ps_gh, Bts, gh_rz, gh_n05)

                def emit_head(p):
                    s, hb = divmod(p, NH)
                    bsl = slice(hb * BH, (hb + 1) * BH)
                    hT = hT0[:, :, bsl] if s == 0 else hidT_v[:, :, bsl, s - 1]
                    psq = ps_q[hb]
                    qt = f"q{hb}"
                    ones_m = sb_ones32[0:1, 0:BH]

                    # hp = Wh2h @ h + bh2h (bias as K=1 row matmul), two psum
                    # tiles of two mt-chunks each; slabs read hp from PSUM.
                    hps = []
                    for half in range(2):
                        ps_hp = psq.tile(
                            [128, 2, BH], f32, tag=qt, name=f"hp{s}_{hb}_{half}"
                        )
                        for k in range(2):
                            mt = 2 * half + k
                            nc.tensor.matmul(
                                ps_hp[:, k, :],
                                sb_hrow[0:1, ts(mt, 128)],
                                ones_m,
                                start=(k == 0),
                                stop=False,
                            )
                            for kt in range(HT):
                                nc.tensor.matmul(
                                    ps_hp[:, k, :],
                                    sb_wh2hT[:, kt, ts(mt, 128)],
                                    hT[:, kt, :],
                                    start=False,
                                    stop=(k == 1 and kt == HT - 1),
                                )
                        hps.append(ps_hp)

                    # B = tanh(hp_{ht2,3}) straight from PSUM, then *(1-2^-11)
                    Bt = sp.tile([128, 2, BH], f16, tag=f"Bt{hb}")
                    nc.scalar.activation(Bt, hps[1], AF.Tanh)
                    Bts = sp.tile([128, 2, BH], f16, tag=f"Bts{hb}")
                    nc.vector.tensor_scalar_mul(Bts, Bt, ASCALE)
                    # hp evac for the Pool adds, emitted BEFORE the customs so
                    # the DVE runs this tiny copy first and the classic-slab
                    # chain isn't starved behind 4.4us of custom slabs
                    hpT01 = sp.tile([128, 2, BH], f16, tag=f"hpT{hb}")
                    nc.vector.tensor_copy(hpT01, hps[0])

                    # gh rz-part accumulates straight into the gi psum tile
                    # (one group spanning head+body; trz later reads PSUM).
                    ps_gi = psq.tile(
                        [128, 2 * HT * BH], f32, tag=qt, name=f"gi{s}_{hb}"
                    )
                    for mt in range(2 * HT):
                        nc.tensor.matmul(
                            ps_gi[:, ts(mt, BH)],
                            sb_grow[0:1, ts(mt, 128)],
                            ones_m,
                            start=(mt == 0),
                            stop=False,
                        )
                    for mt in range(2 * HT):
                        for kt in range(HT):
                            nc.tensor.matmul(
                                ps_gi[:, ts(mt, BH)],
                                sb_whhT[:, kt, ts(mt, 128)],
                                hT[:, kt, :],
                                start=False,
                                stop=False,
                            )
                    # gh n-part (weights and bias pre-scaled by 0.5 host-side)
                    ps_ghn = psq.tile(
                        [128, HT * BH], f32, tag=qt, name=f"ghn{s}_{hb}"
                    )
                    for mt in range(HT):
                        nc.tensor.matmul(
                            ps_ghn[:, ts(mt, BH)],
                            sb_grow[0:1, ts(2 * HT + mt, 128)],
                            ones_m,
                            start=(mt == 0),
                            stop=False,
                        )
                    for mt in range(HT):
                        for kt in range(HT):
                            nc.tensor.matmul(
                                ps_ghn[:, ts(mt, BH)],
                                sb_whhT[:, kt, ts(2 * HT + mt, 128)],
                                hT[:, kt, :],
                                start=False,
                                stop=(mt == HT - 1 and kt == HT - 1),
                            )
                    gh_n05 = sp.tile([128, HT, BH], f16, tag=f"ghn{hb}")
                    nc.scalar.copy(
                        gh_n05,
                        ps_ghn.rearrange("p (m b) -> p m b", b=BH),
                    )

                    # Slabs ht=2,3: fused custom op on DVE (depends only on
                    # Bts -> runs during the previous phase's tail)
                    gc = []
                    for k in range(2):
                        g = gp.tile([128, T * BH], f16, tag=f"gc{hb}", bufs=2)
                        nc.vector._custom_dve(
                            TANH_SUM,
                            out=g.rearrange("p (t b) -> p t b", b=BH),
                            in0=AT_v[:, k, :, bsl],
                            in1=Bts[:, k, :].unsqueeze(1).broadcast_to(
                                [128, T, BH]
                            ),
                            s1=-8.5,
                        )
                        gc.append(g)

                    # Slabs ht=0,1 broadcast-adds (Pool)
                    gt = []
                    for k in range(2):
                        g = gp.tile([128, T * BH], f16, tag=f"ga{hb}_{k}", bufs=2)
                        gv = g.rearrange("p (t b) -> p t b", b=BH)
                        for th in range(2):
                            tsl = slice(th * 128, (th + 1) * 128)
                            nc.gpsimd.tensor_tensor(
                                out=gv[:, tsl, :],
                                in0=fpT_v[:, k, tsl, bsl],
                                in1=hpT01[:, k, :].unsqueeze(1).broadcast_to(
                                    [128, 128, BH]
                                ),
                                op=OP.add,
                            )
                        gt.append(g)

                    head_st[p] = (gc, gt, ps_gi, gh_n05)

                def emit_body(p):
                    s, hb = divmod(p, NH)
                    bsl = slice(hb * BH, (hb + 1) * BH)
                    hT = hT0[:, :, bsl] if s == 0 else hidT_v[:, :, bsl, s - 1]
                    psq = ps_q[hb]
                    qt = f"q{hb}"
                    gc, gt, ps_gi, gh_n05 = head_st.pop(p)

                    # ACT tanh in place over the add outputs, in t-halves so
                    # small ACT ops (Bt/exp) aren't stuck behind 1.9us slabs
                    # and the tt=0 eT bursts can start after the first half
                    for g in gt:
                        h2 = T * BH // 2
                        nc.scalar.activation(g[:, 0:h2], g[:, 0:h2], AF.Tanh)
                        nc.scalar.activation(g[:, h2:], g[:, h2:], AF.Tanh)

                    # e^T psum [128p(t), tt, b] accumulated over all 4 slabs
                    eT = psq.tile([128, TT, BH], f32, tag=qt, name=f"eT{s}_{hb}")
                    slabs = [(gc[0], 2), (gc[1], 3), (gt[0], 0), (gt[1], 1)]
                    for idx, (slab, ht) in enumerate(slabs):
                        slab_v = slab.rearrange("p (t b) -> p t b", b=BH)
                        for tt in range(TT):
                            for b in range(BH):
                                nc.tensor.matmul(
                                    eT[:, tt, b : b + 1],
                                    slab_v[:, tt * 128 : (tt + 1) * 128, b],
                                    sb_wsc[:, ht : ht + 1],
                                    start=(idx == 0 and b == 0 and tt == 0),
                                    stop=(
                                        idx == 3 and b == BH - 1 and tt == TT - 1
                                    ),
                                )

                    expT = sp.tile([128, TT, BH], f16, tag=f"expT{hb}")
                    nc.scalar.activation(expT, eT, AF.Exp)

                    # softmax denominator broadcast to all partitions in one
                    # matmul (ones stationary), then reciprocal
                    ps_sum = psq.tile([128, BH], f32, tag=qt, name=f"sum{s}_{hb}")
                    for tt in range(TT):
                        nc.tensor.matmul(
                            ps_sum,
                            sb_onesq,
                            expT[:, tt, :],
                            start=(tt == 0),
                            stop=(tt == TT - 1),
                        )
                    recip = sp.tile([128, BH], f32, tag=f"rc{hb}")
                    nc.vector.reciprocal(recip, ps_sum)

                    # ctx (one psum tile, normalized in one evac)
                    ctxT = sp.tile([128, CT, BH], f16, tag=f"ctxT{hb}")
                    ps_ctx = psq.tile(
                        [128, CT, BH], f32, tag=qt, name=f"cx{s}_{hb}"
                    )
                    for cc in range(CT):
                        for b in range(BH):
                            bg = hb * BH + b
                            for tt in range(TT):
                                nc.tensor.matmul(
                                    ps_ctx[:, cc, b : b + 1],
                                    sb_featsT[
                                        :,
                                        tt,
                                        bg * C + cc * 128 : bg * C + (cc + 1) * 128,
                                    ],
                                    expT[:, tt, b : b + 1],
                                    start=(cc == 0 and b == 0 and tt == 0),
                                    stop=(
                                        cc == CT - 1
                                        and b == BH - 1
                                        and tt == TT - 1
                                    ),
                                )
                    nc.vector.tensor_tensor(
                        out=ctxT,
                        in0=ps_ctx,
                        in1=recip.unsqueeze(1).broadcast_to([128, CT, BH]),
                        op=OP.mult,
                    )

                    # gi rz-part continues the ps_gi group; n-part separate
                    ps_gin = psq.tile(
                        [128, HT * BH], f32, tag=qt, name=f"gin{s}_{hb}"
                    )
                    ones_m = sb_ones32[0:1, 0:BH]
                    for mt in range(2 * HT):
                        for kt in range(CT):
                            nc.tensor.matmul(
                                ps_gi[:, ts(mt, BH)],
                                sb_wihT[:, kt, ts(mt, 128)],
                                ctxT[:, kt, :],
                                start=False,
                                stop=(mt == 2 * HT - 1 and kt == CT - 1),
                            )
                    for mt in range(HT):
                        nc.tensor.matmul(
                            ps_gin[:, ts(mt, BH)],
                            sb_nrow[0:1, ts(mt, 128)],
                            ones_m,
                            start=(mt == 0),
                            stop=False,
                        )
                    for mt in range(HT):
                        for kt in range(CT):
                            nc.tensor.matmul(
                                ps_gin[:, ts(mt, BH)],
                                sb_wihT[:, kt, ts(2 * HT + mt, 128)],
                                ctxT[:, kt, :],
                                start=False,
                                stop=False,
                            )
                    # + gh_n05 (the linear half of r*hn = 0.5(tr+1)*hn), off
                    # the critical chain since gh_n05 is ready from the head
                    for mt in range(HT):
                        nc.tensor.matmul(
                            ps_gin[:, ts(mt, BH)],
                            sb_ident,
                            gh_n05[:, mt, :],
                            start=False,
                            stop=False,
                        )

                    # Gates. sigmoid(x) = .5 + .5*tanh(x/2); rz from PSUM
                    trz = sp.tile([128, 2 * HT, BH], f32, tag=f"trz{hb}")
                    nc.scalar.activation(
                        trz,
                        ps_gi.rearrange("p (m b) -> p m b", b=BH),
                        AF.Tanh,
                        scale=0.5,
                    )
                    # t2 = tr * (0.5*(gh_n+bhh_n)); the +gh_n05 linear term
                    # is already in ps_gin via ident matmuls
                    t2f = sp.tile([128, HT, BH], f16, tag=f"t2{hb}")
                    nc.gpsimd.tensor_tensor(
                        out=t2f, in0=trz[:, 0:HT, :], in1=gh_n05, op=OP.mult
                    )
                    for mt in range(HT):
                        nc.tensor.matmul(
                            ps_gin[:, ts(mt, BH)],
                            sb_ident,
                            t2f[:, mt, :],
                            start=False,
                            stop=(mt == HT - 1),
                        )
                    n_g = sp.tile([128, HT, BH], f32, tag=f"ng{hb}")
                    nc.scalar.activation(
                        n_g, ps_gin.rearrange("p (m b) -> p m b", b=BH), AF.Tanh
                    )
                    # h' = n + 0.5*(tz+1)*(h-n), all Pool TensorTensor
                    zsa = sp.tile([128, HT, BH], f32, tag=f"zsa{hb}")
                    nc.gpsimd.tensor_tensor(
                        out=zsa,
                        in0=trz[:, HT : 2 * HT, :],
                        in1=sb_half.unsqueeze(1).broadcast_to([128, HT, BH]),
                        op=OP.mult,
                    )
                    zs = sp.tile([128, HT, BH], f32, tag=f"zs{hb}")
                    nc.gpsimd.tensor_tensor(
                        out=zs,
                        in0=zsa,
                        in1=sb_half.unsqueeze(1).broadcast_to([128, HT, BH]),
                        op=OP.add,
                    )
                    d = sp.tile([128, HT, BH], f32, tag=f"d{hb}")
                    nc.gpsimd.tensor_sub(d, hT, n_g)
                    m7 = sp.tile([128, HT, BH], f32, tag=f"m7{hb}")
                    nc.gpsimd.tensor_mul(m7, zs, d)
                    nc.gpsimd.tensor_add(hidT_v[:, :, bsl, s], n_g, m7)

                emit_head(0)
                for p in range(NPH):
                    emit_body(p)
                    if p == 0:
                        emit_head(1)
                    if p + 2 < NPH:
                        emit_head(p + 2)

                # ---- Epilogue: probs = hiddens @ Wgen.T + bgen ----
                for rt in range(CT):
                    ps_pr = ps_q0.tile([128, CLS], f32, tag="q0", name=f"pr{rt}")
                    for kt in range(HT):
                        nc.tensor.matmul(
                            ps_pr,
                            sb_hidT[:, kt, ts(rt, 128)],
                            sb_wgenT[:, kt, :],
                            start=(kt == 0),
                            stop=False,
                        )
                    nc.tensor.matmul(
                        ps_pr, sb_ones128, sb_bgen, start=False, stop=True
                    )
                    pr = sp.tile([128, CLS], f32, tag="pr_out")
                    nc.vector.tensor_copy(pr, ps_pr)
                    nc.gpsimd.dma_start(probs_d.ap()[ts(rt, 128)], pr)

    nc.compile()
    return nc


def make_in_maps(feats, Wi2h, Wh2h, bh2h, Wscore, Wih, Whh, bih, bhh, Wgen, bgen):
    """Host-side prep: cast, transpose weights, shard feats over batch."""
    f16 = np.float16
    f32 = np.float32
    feats = np.asarray(feats, f32)
    wsc = np.ascontiguousarray(
        np.asarray(Wscore, np.float64)[0].reshape(HT, 128).T
    )
    wsc[:, 2:4] /= KREC
    bih = np.asarray(bih, f32)
    bhh = np.asarray(bhh, f32)
    grow = np.concatenate([(bih + bhh)[: 2 * H], 0.5 * bhh[2 * H :]]).astype(f32)
    common = {
        "wi2hT": np.ascontiguousarray(np.asarray(Wi2h).T).astype(f16).reshape(CT, 128, H),
        "wh2hT": np.ascontiguousarray(np.asarray(Wh2h).T).astype(f32).reshape(HT, 128, H),
        "whhT": np.ascontiguousarray(
            np.asarray(Whh).T * np.concatenate([np.ones(2 * H), np.full(H, 0.5)])
        ).astype(f32).reshape(HT, 128, G3),
        "wihT": np.ascontiguousarray(np.asarray(Wih).T).astype(f16).reshape(CT, 128, G3),
        "wgenT": np.ascontiguousarray(np.asarray(Wgen).T).astype(f32).reshape(HT, 128, CLS),
        "wsc": wsc.astype(f16),
        "hrow": np.asarray(bh2h, f32).reshape(1, H),
        "grow": grow.reshape(1, G3),
        "nrow": bih[2 * H :].reshape(1, H).astype(f32),
        "ident": np.eye(128, dtype=f16),
        "bgen": np.asarray(bgen, f32).astype(f16).reshape(1, CLS),
    }
    in_maps = []
    for i in range(NCORES):
        sl = slice(i * B, (i + 1) * B)
        fsh = feats[:, sl, :]  # [512, 16, 256]
        m = dict(common)
        # t-major free layout (col = t*16 + b) for the broadcast-adds
        m["feats"] = (
            np.ascontiguousarray(fsh.transpose(0, 2, 1)).astype(f16).reshape(CT, 128, T * B)
        )
        m["featsT"] = (
            np.ascontiguousarray(fsh.transpose(2, 1, 0)).astype(f16).reshape(TT, 128, B * C)
        )
        in_maps.append(m)
    return in_maps


def _get_nc(n_steps=S):
    k = f"nc{n_steps}"
    if k not in _CACHE:
        _CACHE[k] = build_nc(n_steps)
    return _CACHE[k]


def kernel(
    feats,
    text_length,
    Wi2h,
    Wh2h,
    bh2h,
    Wscore,
    Wih,
    Whh,
    bih,
    bhh,
    Wgen,
    bgen,
    **_ignored,
):
    from concourse import bass_utils

    nc = _get_nc()
    in_maps = make_in_maps(
        feats, Wi2h, Wh2h, bh2h, Wscore, Wih, Whh, bih, bhh, Wgen, bgen
    )
    res = bass_utils.run_bass_kernel_spmd(nc, in_maps, core_ids=list(range(NCORES)))
    out = np.concatenate([r["probs"] for r in res.results], axis=0)
    return out.astype(np.float32)
